# revision 1
# baseline (speedup 1.0000x reference)
"""DIFFormer (linear attention + GCN) Trainium2 kernel, 8-core SPMD.

Self-contained: only numpy + concourse imports. Hardcoded shapes for
N=50000, E=800000, IN=512, HC=256, H=4, L=2, OUT=64.

Sharding: nodes across 8 cores (6272 padded rows each). Linear attention
uses all-reduced kvs/ks/vs stats; GCN uses an all-gathered (dinv-scaled)
xw table, per-edge dma_gather by destination chunk, and one-hot
selection-matrix matmuls for the segmented sum.
"""

import time
import numpy as np

import jax
from jax.sharding import Mesh, PartitionSpec
from jax.experimental.shard_map import shard_map

import concourse.bass as bass
import concourse.bacc as bacc
import concourse.tile as tile
import concourse.mybir as mybir
from concourse import bass2jax, library_config
from concourse.bass2jax import _bass_exec_p, install_neuronx_cc_hook

# ---------------------------------------------------------------- constants
N, E, IN, HC, H, L, OUT = 50000, 800000, 512, 256, 4, 2, 64
D2 = 2 * HC                       # 512
ALPHA, EPS = 0.5, 1e-5
NCORES = 8
NT = 49                           # node tiles per core
NLOC = NT * 128                   # 6272
NP = NCORES * NLOC                # 50176
HALF = NP // 2                    # legacy
SPLIT1 = 3200                     # local rows in table1 (tiles 0-24)
SPLIT2 = NLOC - SPLIT1            # 3072 (tiles 25-48)
TB1 = NCORES * SPLIT1             # 25600 rows, int16-safe
TB2 = NCORES * SPLIT2             # 24576 rows, int16-safe
P = 128
BF = np.dtype("bfloat16")
F32 = mybir.dt.float32
BF16 = mybir.dt.bfloat16
I16 = mybir.dt.int16
AF = mybir.ActivationFunctionType
ALU = mybir.AluOpType
PAD_SENTINEL = 300.0              # dst-slot value for padded edge slots


def _roundup(x, m):
    return (x + m - 1) // m * m


def _wrap_idx(arr):
    """int16 index array (len multiple of 16) -> [128, len/16] wrapped layout:
    idx j at partition j%16, col j//16, replicated across 8 Q7 cores."""
    a = arr.reshape(-1, 16).T  # [16, len/16]
    return np.tile(a, (8, 1)).astype(np.int16)


def _host_prep(x, edge_index, fc0_w, fc0_b, wq, bq, wk, bk, wv, bv,
               gcn_w, gcn_b, bn_gamma, bn_beta, bn_mean, bn_var,
               fc_out_w, fc_out_b):
    """Build all per-core device input arrays + compile-time metadata."""
    meta = {}

    # ---- edges: append self-loops, degree norm, sort by dst
    src = np.asarray(edge_index[0], dtype=np.int64)
    dst = np.asarray(edge_index[1], dtype=np.int64)
    src_all = np.concatenate([src, np.arange(N, dtype=np.int64)])
    dst_all = np.concatenate([dst, np.arange(N, dtype=np.int64)])
    deg = np.bincount(dst_all, minlength=NP).astype(np.float64)
    dinv = 1.0 / np.sqrt(np.maximum(deg, 1.0))
    dinv = dinv.astype(np.float32)          # [NP]; pad nodes -> 1.0 (deg 0)

    order = np.argsort(dst_all, kind="stable")
    s_s, s_d = src_all[order], dst_all[order]

    # per (core, chunk) edge lists split into src halves
    nchunks = NT
    cnt = np.zeros((NCORES, nchunks, 2), dtype=np.int64)
    # bucket boundaries by dst
    bounds = np.searchsorted(s_d, np.arange(0, NP + 1, 128))
    lists = [[None] * nchunks for _ in range(NCORES)]
    for r in range(NCORES):
        for c in range(nchunks):
            g = r * NT + c
            lo, hi = bounds[g], bounds[g + 1]
            es, ed = s_s[lo:hi], s_d[lo:hi]
            rsrc = es // NLOC
            jloc = es % NLOC
            a_mask = jloc < SPLIT1
            ea = (rsrc[a_mask] * SPLIT1 + jloc[a_mask]).astype(np.int64)
            da = ed[a_mask]
            eb = (rsrc[~a_mask] * SPLIT2 + (jloc[~a_mask] - SPLIT1)).astype(np.int64)
            db = ed[~a_mask]
            lists[r][c] = (ea, da, eb, db)
            cnt[r, c, 0] = len(ea)
            cnt[r, c, 1] = len(eb)

    capA = np.maximum(128, _roundup(cnt[:, :, 0].max(axis=0), 128))  # [nchunks]
    capB = np.maximum(128, _roundup(cnt[:, :, 1].max(axis=0), 128))
    nkt = (capA + capB) // 128                                       # [nchunks]
    meta["capA"], meta["capB"], meta["nkt"] = capA, capB, nkt
    meta["idx_cols"] = int((capA.sum() + capB.sum()) // 16)
    meta["nkt_tot"] = int(nkt.sum())

    # chunk pairs: gathers merged per pair (one A-gather + one B-gather)
    pairs = [list(range(g, min(g + 2, nchunks))) for g in range(0, nchunks, 2)]
    meta["pairs"] = pairs
    idx_all = np.zeros((NCORES, 128, meta["idx_cols"]), dtype=np.int16)
    dst_cols = np.full((NCORES, 128, meta["nkt_tot"]), PAD_SENTINEL,
                       dtype=np.float32)
    icol = 0
    kcol = 0
    meta["pair_icolA"] = {}
    meta["pair_icolB"] = {}
    meta["chunk_kcol"] = [0] * nchunks
    for pi, pc in enumerate(pairs):
        pca = int(sum(capA[c] for c in pc))
        pcb = int(sum(capB[c] for c in pc))
        meta["pair_icolA"][pi] = icol
        meta["pair_icolB"][pi] = icol + pca // 16
        for r in range(NCORES):
            ia = np.zeros(pca, dtype=np.int16)
            ib = np.zeros(pcb, dtype=np.int16)
            oa = ob = 0
            for c in pc:
                ea, da, eb, db = lists[r][c]
                ia[oa:oa + len(ea)] = ea
                ib[ob:ob + len(eb)] = eb
                oa += int(capA[c])
                ob += int(capB[c])
            idx_all[r][:, icol:icol + pca // 16] = _wrap_idx(ia)
            idx_all[r][:, icol + pca // 16:icol + (pca + pcb) // 16] = \
                _wrap_idx(ib)
        icol += (pca + pcb) // 16
        for c in pc:
            meta["chunk_kcol"][c] = kcol
            ca, cb = int(capA[c]), int(capB[c])
            for r in range(NCORES):
                ea, da, eb, db = lists[r][c]
                dloc = np.full(ca + cb, PAD_SENTINEL, dtype=np.float32)
                dloc[:len(da)] = (da - (r * NLOC + c * 128)).astype(np.float32)
                dloc[ca:ca + len(db)] = \
                    (db - (r * NLOC + c * 128)).astype(np.float32)
                dst_cols[r][:, kcol:kcol + (ca + cb) // 128] = \
                    dloc.reshape(-1, 128).T
            kcol += (ca + cb) // 128
    # host-built one-hot selection tiles: S[p, j, x] = (dst_cols[p,j] == x)
    s_hosts = []
    xr = np.arange(128, dtype=np.float32)
    for r in range(NCORES):
        sh_ = (dst_cols[r][:, :, None] == xr).astype(np.float32)
        sh_ = sh_.astype(np.dtype("float8_e4m3fn"))
        s_hosts.append(np.ascontiguousarray(sh_.reshape(128, -1)))


    # ---- per-core node data
    xpad = np.zeros((NP, IN), dtype=np.float32)
    xpad[:N] = np.asarray(x, dtype=np.float32)
    mask = np.zeros((NP,), dtype=np.float32)
    mask[:N] = 1.0
    dinv_m = dinv * mask

    per_core = []
    for r in range(NCORES):
        sl = slice(r * NLOC, (r + 1) * NLOC)
        xs = xpad[sl]                                    # [6272, 512]
        # XT tiles layout [128, NT, 4, 128]: [p,t,k,j] = x[t*128+j, k*128+p]
        xt = np.ascontiguousarray(
            xs.reshape(NT, 128, 4, 128).transpose(3, 0, 2, 1)).astype(BF)
        d = {
            "xt": xt.reshape(128, NT * 4 * 128),
            "mask": mask[sl].reshape(NT, 128).T.copy(),          # [128, NT]
            "dinv": dinv_m[sl].reshape(NT, 128).T.copy(),        # [128, NT]
            "idx_all": idx_all[r],
            "s_all": s_hosts[r],
        }
        per_core.append(d)

    # ---- weights / constants (shared across cores)
    def rhs_layout(w):
        # [D2, W] -> [128, 4, W] with [p,k,n] = w[k*128+p, n]
        wv_ = np.asarray(w, dtype=np.float32)
        return np.ascontiguousarray(
            wv_.reshape(4, 128, -1).transpose(1, 0, 2)).astype(BF)

    bn_gamma = np.asarray(bn_gamma, np.float32)
    bn_beta = np.asarray(bn_beta, np.float32)
    bn_mean = np.asarray(bn_mean, np.float32)
    bn_var = np.asarray(bn_var, np.float32)
    fc0_b = np.asarray(fc0_b, np.float32)
    gcn_b = np.asarray(gcn_b, np.float32)

    scale = bn_gamma / np.sqrt(bn_var + EPS)             # [L+1, D2]
    shift = bn_beta - bn_mean * scale
    # BN0 applies to x@W + fc0_b
    sc0, sh0 = scale[0], shift[0] + scale[0] * fc0_b
    bnscale = [sc0.astype(np.float32)]
    bnshift = [sh0.astype(np.float32)]
    for i in range(L):
        sc = ALPHA * scale[i + 1]
        sh = shift[i + 1].copy()
        sh[HC:] += ALPHA * scale[i + 1][HC:] * gcn_b[i]
        bnscale.append(sc.astype(np.float32))
        bnshift.append(sh.astype(np.float32))

    shared = {
        "fc0w": rhs_layout(fc0_w).reshape(128, 4 * D2),
        "fcoutw": rhs_layout(fc_out_w).reshape(128, 4 * OUT),
        "eps": np.full((128, 1), 1e-12, dtype=np.float32),
        "eps16": np.full((128, 1), 16e-12, dtype=np.float32),
        "onescol": np.ones((128, 1), dtype=np.float32).astype(BF),
        "onesrow": np.ones((1, 128), dtype=np.float32).astype(BF),
    }
    for i in range(L):
        wkv = np.concatenate([np.asarray(wk[i]), np.asarray(wv[i])], axis=1)
        shared[f"wkv{i}"] = rhs_layout(wkv).reshape(128, 4 * 2048)
        shared[f"wq{i}"] = rhs_layout(wq[i]).reshape(128, 4 * 1024)
        shared[f"gcnw{i}"] = rhs_layout(gcn_w[i]).reshape(128, 4 * HC)
    for j in range(L + 1):
        shared[f"bnsc{j}"] = np.tile(bnscale[j], (128, 1)).astype(BF)
        shared[f"bnsh{j}"] = np.tile(bnshift[j], (128, 1)).astype(BF)

    meta["qkv_bias"] = bool(np.any(np.asarray(bq)) or np.any(np.asarray(bk))
                            or np.any(np.asarray(bv)))
    if meta["qkv_bias"]:
        for i in range(L):
            shared[f"bkv{i}"] = np.concatenate(
                [np.asarray(bk[i]), np.asarray(bv[i])]).reshape(1, 2048).astype(BF)
            shared[f"bq{i}"] = np.asarray(bq[i]).reshape(1, 1024).astype(BF)
    meta["out_bias"] = bool(np.any(np.asarray(fc_out_b)))
    if meta["out_bias"]:
        shared["fcoutb"] = np.tile(np.asarray(fc_out_b, np.float32),
                                   (128, 1))

    in_maps = []
    for r in range(NCORES):
        m = dict(per_core[r])
        m.update(shared)
        in_maps.append(m)
    return in_maps, meta


# ------------------------------------------------------------- program build
def _build_nc(meta, debug=False, single=False):
    nc = bacc.Bacc("TRN2", target_bir_lowering=False, debug=False,
                   num_devices=1 if single else NCORES, num_swdge_queues=4)

    # ---- external inputs
    T = {}
    T["xt"] = nc.dram_tensor("xt", [128, NT * 4 * 128], BF16, kind="ExternalInput")
    T["mask"] = nc.dram_tensor("mask", [128, NT], F32, kind="ExternalInput")
    T["dinv"] = nc.dram_tensor("dinv", [128, NT], F32, kind="ExternalInput")
    T["idx_all"] = nc.dram_tensor("idx_all", [128, meta["idx_cols"]], I16,
                                  kind="ExternalInput")
    T["s_all"] = nc.dram_tensor("s_all", [128, meta["nkt_tot"] * 128],
                                mybir.dt.float8e4, kind="ExternalInput")
    T["fc0w"] = nc.dram_tensor("fc0w", [128, 4 * D2], BF16, kind="ExternalInput")
    T["fcoutw"] = nc.dram_tensor("fcoutw", [128, 4 * OUT], BF16,
                                 kind="ExternalInput")
    T["eps"] = nc.dram_tensor("eps", [128, 1], F32, kind="ExternalInput")
    T["eps16"] = nc.dram_tensor("eps16", [128, 1], F32, kind="ExternalInput")
    T["onescol"] = nc.dram_tensor("onescol", [128, 1], BF16, kind="ExternalInput")
    T["onesrow"] = nc.dram_tensor("onesrow", [1, 128], BF16, kind="ExternalInput")
    for i in range(L):
        T[f"wkv{i}"] = nc.dram_tensor(f"wkv{i}", [128, 4 * 2048], BF16,
                                      kind="ExternalInput")
        T[f"wq{i}"] = nc.dram_tensor(f"wq{i}", [128, 4 * 1024], BF16,
                                     kind="ExternalInput")
        T[f"gcnw{i}"] = nc.dram_tensor(f"gcnw{i}", [128, 4 * HC], BF16,
                                       kind="ExternalInput")
        if meta["qkv_bias"]:
            T[f"bkv{i}"] = nc.dram_tensor(f"bkv{i}", [1, 2048], BF16,
                                          kind="ExternalInput")
            T[f"bq{i}"] = nc.dram_tensor(f"bq{i}", [1, 1024], BF16,
                                         kind="ExternalInput")
    for j in range(L + 1):
        T[f"bnsc{j}"] = nc.dram_tensor(f"bnsc{j}", [128, D2], BF16,
                                       kind="ExternalInput")
        T[f"bnsh{j}"] = nc.dram_tensor(f"bnsh{j}", [128, D2], BF16,
                                       kind="ExternalInput")
    if meta["out_bias"]:
        T["fcoutb"] = nc.dram_tensor("fcoutb", [128, OUT], F32,
                                     kind="ExternalInput")

    out_d = nc.dram_tensor("out", [NLOC, OUT], F32, kind="ExternalOutput")

    # ---- internal DRAM
    xw_in1 = nc.dram_tensor("xw_in1", [SPLIT1, HC], BF16, kind="Internal")
    xw_in2 = nc.dram_tensor("xw_in2", [SPLIT2, HC], BF16, kind="Internal")
    xw_tbl1 = nc.dram_tensor("xw_tbl1", [TB1, HC], BF16, kind="Internal",
                             addr_space="Shared")
    xw_tbl2 = nc.dram_tensor("xw_tbl2", [TB2, HC], BF16, kind="Internal",
                             addr_space="Shared")
    CCK = 4 * 128 * 514           # kvs region floats
    cc_in = nc.dram_tensor("cc_in", [CCK + 128 * 8], F32, kind="Internal")
    cc_out = nc.dram_tensor("cc_out", [CCK + 128 * 8], F32, kind="Internal",
                            addr_space="Shared")
    dbg = {}
    if debug:
        for nm, shp in [("h0_dbg", [128, NT * D2]), ("h1_dbg", [128, NT * D2]),
                        ("x1_dbg", [128, NT * HC]), ("x2_dbg", [128, NT * HC]),
                        ("cc_dbg", [CCK + 128 * 8])]:
            dbg[nm] = nc.dram_tensor(nm, shp, F32 if nm == "cc_dbg" else BF16,
                                     kind="ExternalOutput")

    capA, capB, nkt = meta["capA"], meta["capB"], meta["nkt"]
    max_nkt = int(nkt.max())

    with tile.TileContext(nc) as tc:
        with tc.tile_pool(name="const", bufs=1) as cp, \
             tc.tile_pool(name="ht", bufs=4) as htp, \
             tc.tile_pool(name="stage", bufs=2) as stp, \
             tc.tile_pool(name="scratch", bufs=6) as scp, \
             tc.tile_pool(name="small", bufs=8) as smp, \
             tc.tile_pool(name="gpool", bufs=2) as gp, \
             tc.tile_pool(name="spool", bufs=2) as sp_, \
             tc.tile_pool(name="ps", bufs=1, space="PSUM") as ps:

            nc.gpsimd.load_library(library_config.mlp)

            # PSUM: 8 banks juggled manually via tags psb0..psb7.
            _cnt = [0, 0]

            _projtags = [["psb6", "psb7"]]

            def proj_tile():
                tags = _projtags[0]
                t = ps.tile([128, 512], F32, space="PSUM",
                            tag=tags[_cnt[0] % len(tags)],
                            name=f"proj{_cnt[0]}")
                _cnt[0] += 1
                return t

            def gcn_tile():
                t = ps.tile([128, HC], F32, space="PSUM",
                            tag=f"psb{2 + _cnt[1] % 2}",
                            name=f"gcn{_cnt[1]}")
                _cnt[1] += 1
                return t

            _ndcnt = [0]

            def nd_tile(name):
                t = ps.tile([128, 257], F32, space="PSUM",
                            tag=f"psb{_ndcnt[0] % 2}", name=name)
                _ndcnt[0] += 1
                return t

            # ---- load constants
            def cload(name, shape, dtype):
                t = cp.tile(shape, dtype, tag=name)
                nc.sync.dma_start(t[:], T[name][:])
                return t

            eps = cload("eps", [128, 1], F32)
            eps16 = cload("eps16", [128, 1], F32)
            onescol = cload("onescol", [128, 1], BF16)
            onesrow = cload("onesrow", [1, 128], BF16)
            maskc = cload("mask", [128, NT], F32)
            dinvc = cload("dinv", [128, NT], F32)
            fc0w = cload("fc0w", [128, 4 * D2], BF16)
            fcoutw = cload("fcoutw", [128, 4 * OUT], BF16)
            gcnw = [cload(f"gcnw{i}", [128, 4 * HC], BF16) for i in range(L)]
            bnsc = [cload(f"bnsc{j}", [128, D2], BF16) for j in range(L + 1)]
            bnsh = [cload(f"bnsh{j}", [128, D2], BF16) for j in range(L + 1)]
            bkv = bq_ = None
            if meta["qkv_bias"]:
                bkv = [cload(f"bkv{i}", [1, 2048], BF16) for i in range(L)]
                bq_ = [cload(f"bq{i}", [1, 1024], BF16) for i in range(L)]
            maskbf = cp.tile([128, NT], BF16, tag="maskbf")
            nc.vector.tensor_copy(maskbf[:], maskc[:])

            # persistent h tiles
            Ht = [cp.tile([128, D2], BF16, tag=f"H{t}", name=f"H{t}")
                  for t in range(NT)]

            # ---------------- phase 0: h0 = relu(bn0(x @ fc0_w))
            for t in range(NT):
                xt_t = htp.tile([128, 4, 128], BF16, tag="xt")
                nc.sync.dma_start(
                    xt_t[:], T["xt"][:, t * 512:(t + 1) * 512].rearrange(
                        "p (k j) -> p k j", k=4))
                h0p = proj_tile()
                for k in range(4):
                    nc.tensor.matmul(h0p[:], lhsT=xt_t[:, k, :],
                                     rhs=fc0w[:, k * D2:(k + 1) * D2],
                                     start=(k == 0), stop=(k == 3))
                w1 = stp.tile([128, D2], BF16, tag="epi1")
                nc.vector.tensor_tensor(w1[:], h0p[:], bnsc[0][:], op=ALU.mult)
                w2 = stp.tile([128, D2], BF16, tag="epi2")
                nc.vector.tensor_tensor(w2[:], w1[:], bnsh[0][:], op=ALU.add)
                nc.vector.tensor_scalar(Ht[t][:], w2[:], 0.0,
                                        maskc[:, t:t + 1], op0=ALU.max,
                                        op1=ALU.mult)
            if debug:
                for t in range(NT):
                    nc.sync.dma_start(
                        dbg["h0_dbg"][:, t * D2:(t + 1) * D2], Ht[t][:])

            # ---------------- layers
            for li in range(L):
                # per-layer weights into shared slots
                wkv_t = cp.tile([128, 4 * 2048], BF16, tag="wkv",
                                name=f"wkv_l{li}")
                nc.sync.dma_start(wkv_t[:], T[f"wkv{li}"][:])
                wq_t = cp.tile([128, 4 * 1024], BF16, tag="wq",
                               name=f"wq_l{li}")
                nc.sync.dma_start(wq_t[:], T[f"wq{li}"][:])
                # ---- pass 1: k,v,xw projections; kvs/ks/vs accumulation
                kvsP = [ps.tile([128, 512], F32, space="PSUM",
                                tag=f"psb{h}", name=f"kvs{li}_{h}")
                        for h in range(H)]
                ksvsP = ps.tile([128, 16], F32, space="PSUM", tag="psb4",
                                name=f"ksvs{li}")
                ksP = ksvsP[:, 0:8]
                vsP = ksvsP[:, 8:16]
                _projtags[0] = ["psb5", "psb6", "psb7"]

                prev_acc = None

                def _emit_acc(kh_, vb_, t_):
                    for hh in range(H):
                        for half in range(2):
                            lhs = kh_[:, hh * HC + half * 128:
                                      hh * HC + half * 128 + 128]
                            nc.tensor.matmul(
                                kvsP[hh][:, half * HC:(half + 1) * HC],
                                lhsT=lhs, rhs=vb_[:, hh * HC:(hh + 1) * HC],
                                start=(t_ == 0), stop=(t_ == NT - 1))
                            nc.tensor.matmul(
                                ksP[:, 2 * hh + half:2 * hh + half + 1],
                                lhsT=lhs, rhs=maskbf[:, t_:t_ + 1],
                                start=(t_ == 0), stop=(t_ == NT - 1))
                            vlhs = vb_[:, hh * HC + half * 128:
                                       hh * HC + half * 128 + 128]
                            nc.tensor.matmul(
                                vsP[:, 2 * hh + half:2 * hh + half + 1],
                                lhsT=vlhs, rhs=maskbf[:, t_:t_ + 1],
                                start=(t_ == 0), stop=(t_ == NT - 1))

                for t in range(NT):
                    ht = htp.tile([128, 4, 128], BF16, tag="ht")
                    nc.sync.dma_start_transpose(ht[:], Ht[t][:])
                    khat = stp.tile([128, 1024], BF16, tag="khat", bufs=3)
                    for nb in range(2):   # k chunks
                        kvp = proj_tile()
                        for k in range(4):
                            nc.tensor.matmul(
                                kvp[:], lhsT=ht[:, k, :],
                                rhs=wkv_t[:, k * 2048 + nb * 512:
                                            k * 2048 + (nb + 1) * 512],
                                start=(k == 0), stop=(k == 3) and bkv is None)
                        if bkv is not None:
                            nc.tensor.matmul(
                                kvp[:], lhsT=onesrow[:],
                                rhs=bkv[li][:, nb * 512:(nb + 1) * 512],
                                start=False, stop=True)
                        ssk = smp.tile([128, 2], F32, tag="ssk")
                        for h2 in range(2):
                            sq = scp.tile([128, HC], BF16, tag="sq")
                            nc.scalar.activation(
                                sq[:], kvp[:, h2 * HC:(h2 + 1) * HC],
                                AF.Square, accum_out=ssk[:, h2:h2 + 1])
                        nrm = smp.tile([128, 2], F32, tag="nrmk")
                        nc.scalar.activation(nrm[:], ssk[:], AF.Sqrt,
                                             bias=eps[:, :1])
                        rskm = smp.tile([128, 2], F32, tag="rskm")
                        nc.vector.reciprocal(rskm[:], nrm[:])
                        nc.vector.tensor_scalar(rskm[:], rskm[:],
                                                maskc[:, t:t + 1], None,
                                                op0=ALU.mult)
                        for h2 in range(2):
                            hh = nb * 2 + h2
                            nc.vector.tensor_scalar(
                                khat[:, hh * HC:(hh + 1) * HC],
                                kvp[:, h2 * HC:(h2 + 1) * HC],
                                rskm[:, h2:h2 + 1], None, op0=ALU.mult)
                    vsb = stp.tile([128, 1024], BF16, tag="vsb", bufs=3)
                    for nb in range(2, 4):  # v chunks
                        kvp = proj_tile()
                        for k in range(4):
                            nc.tensor.matmul(
                                kvp[:], lhsT=ht[:, k, :],
                                rhs=wkv_t[:, k * 2048 + nb * 512:
                                            k * 2048 + (nb + 1) * 512],
                                start=(k == 0), stop=(k == 3) and bkv is None)
                        if bkv is not None:
                            nc.tensor.matmul(
                                kvp[:], lhsT=onesrow[:],
                                rhs=bkv[li][:, nb * 512:(nb + 1) * 512],
                                start=False, stop=True)
                        nc.vector.tensor_copy(
                            vsb[:, (nb - 2) * 512:(nb - 1) * 512], kvp[:])
                    # kvs/ks/vs accumulation: software-pipelined one tile
                    # behind the projections so the PE stream never waits on
                    # this tile's normalize chain.
                    if prev_acc is not None:
                        _emit_acc(*prev_acc)
                    prev_acc = (khat, vsb, t)
                    # xw projection + dinv scale -> table shard
                    xwp = proj_tile()
                    for k in range(4):
                        nc.tensor.matmul(
                            xwp[:, 0:HC], lhsT=ht[:, k, :],
                            rhs=gcnw[li][:, k * HC:(k + 1) * HC],
                            start=(k == 0), stop=(k == 3))
                    xws = stp.tile([128, HC], BF16, tag="xws")
                    nc.vector.tensor_scalar(xws[:], xwp[:, 0:HC],
                                            dinvc[:, t:t + 1], None,
                                            op0=ALU.mult)
                    if t < 25:
                        nc.sync.dma_start(
                            xw_in1[t * 128:(t + 1) * 128, :], xws[:])
                    else:
                        nc.sync.dma_start(
                            xw_in2[(t - 25) * 128:(t - 24) * 128, :], xws[:])
                    if t == 24 and not single:
                        nc.gpsimd.collective_compute(
                            "AllGather", ALU.bypass,
                            replica_groups=[list(range(NCORES))],
                            ins=[xw_in1[:]], outs=[xw_tbl1[:]])

                if prev_acc is not None:
                    _emit_acc(*prev_acc)
                    prev_acc = None

                # ---- flush kvs/ks/vs to cc_in, AllReduce; AllGather xw
                cc_kvs = cc_in[:CCK].rearrange("(h p c) -> h p c", h=4, p=128)
                cc_vs = cc_in[CCK:].rearrange("(p c) -> p c", p=128)
                cco_kvs = cc_out[:CCK].rearrange("(h p c) -> h p c", h=4, p=128)
                cco_vs = cc_out[CCK:].rearrange("(p c) -> p c", p=128)
                for hh in range(H):
                    stg = stp.tile([128, 514], F32, tag="ccstage")
                    for half in range(2):
                        nc.vector.tensor_copy(
                            stg[:, half * 257:half * 257 + 256],
                            kvsP[hh][:, half * HC:(half + 1) * HC])
                        nc.vector.tensor_copy(
                            stg[:, half * 257 + 256:half * 257 + 257],
                            ksP[:, 2 * hh + half:2 * hh + half + 1])
                    nc.sync.dma_start(cc_kvs[hh], stg[:])
                vstg = stp.tile([128, 8], F32, tag="vstage")
                nc.vector.tensor_copy(vstg[:], vsP[:])
                nc.sync.dma_start(cc_vs, vstg[:])
                pair_tiles = {}
                npairs = len(meta["pairs"])

                def issue_pair_gathers(pi2, part):
                    pc2 = meta["pairs"][pi2]
                    pca2 = int(sum(capA[c] for c in pc2))
                    pcb2 = int(sum(capB[c] for c in pc2))
                    ioA2 = meta["pair_icolA"][pi2]
                    if part == 0:
                        idxp2 = sp_.tile([128, (pca2 + pcb2) // 16], I16,
                                         tag="idxp", bufs=3,
                                         name=f"idxp_{li}_{pi2}")
                        nc.sync.dma_start(
                            idxp2[:],
                            T["idx_all"][:, ioA2:ioA2 + (pca2 + pcb2) // 16])
                        GA2 = gp.tile([128, pca2 // 128, HC], BF16, tag="GA",
                                      name=f"GA_{li}_{pi2}")
                        nc.gpsimd.dma_gather(
                            GA2[:], xw_tbl1[:], idxp2[:, 0:pca2 // 16],
                            pca2, pca2, HC, single_packet=False,
                            queue_num=(2 * pi2) % 4)
                        pair_tiles[pi2] = [GA2, None, idxp2]
                    else:
                        idxp2 = pair_tiles[pi2][2]
                        GB2 = gp.tile([128, pcb2 // 128, HC], BF16, tag="GB",
                                      name=f"GB_{li}_{pi2}")
                        nc.gpsimd.dma_gather(
                            GB2[:], xw_tbl2[:],
                            idxp2[:, pca2 // 16:(pca2 + pcb2) // 16],
                            pcb2, pcb2, HC, single_packet=False,
                            queue_num=(2 * pi2 + 1) % 4)
                        pair_tiles[pi2][1] = GB2

                # A-gathers of the first pairs depend only on AG#1 -> issue
                # them before the blocking AG#2/AR on the gpsimd queue.
                issue_pair_gathers(0, 0)
                if npairs > 1:
                    issue_pair_gathers(1, 0)
                if single:
                    nc.sync.dma_start(cc_out[:], cc_in[:])
                    for _rr in range(NCORES):
                        nc.sync.dma_start(
                            xw_tbl1[_rr * SPLIT1:(_rr + 1) * SPLIT1, :],
                            xw_in1[:])
                        nc.sync.dma_start(
                            xw_tbl2[_rr * SPLIT2:(_rr + 1) * SPLIT2, :],
                            xw_in2[:])
                else:
                    nc.gpsimd.collective_compute(
                        "AllGather", ALU.bypass,
                        replica_groups=[list(range(NCORES))],
                        ins=[xw_in2[:]], outs=[xw_tbl2[:]])
                issue_pair_gathers(0, 1)
                if npairs > 1:
                    issue_pair_gathers(1, 1)
                if not single:
                    nc.gpsimd.collective_compute(
                        "AllReduce", ALU.add,
                        replica_groups=[list(range(NCORES))],
                        ins=[cc_in[:]], outs=[cc_out[:]])
                if debug:
                    nc.sync.dma_start(dbg["cc_dbg"][:], cc_out[:])

                # ---- load reduced stats: rhs tiles (0.25-scaled kvs, ks col)
                kvs_rhs = []
                for hh in range(H):
                    row = []
                    for half in range(2):
                        f32t = stp.tile([128, 257], F32, tag="ccload")
                        nc.sync.dma_start(
                            f32t[:], cco_kvs[hh][:, half * 257:(half + 1) * 257])
                        bft = cp.tile([128, 257], BF16, tag=f"kvsr{hh}_{half}",
                                       name=f"kvsr{li}_{hh}_{half}")
                        nc.vector.tensor_scalar(bft[:, 0:256], f32t[:, 0:256],
                                                0.25, None, op0=ALU.mult)
                        nc.vector.tensor_copy(bft[:, 256:257], f32t[:, 256:257])
                        row.append(bft)
                    kvs_rhs.append(row)
                vs_rhs = cp.tile([1, H, 257], BF16, tag="vsr", name=f"vsr{li}")
                nc.vector.memset(vs_rhs[:], 0)
                for hh in range(H):
                    for half in range(2):
                        vrow = stp.tile([1, 128], F32, tag="vsload")
                        nc.sync.dma_start(
                            vrow[:], cco_vs[:, 2 * hh + half:2 * hh + half + 1]
                            .rearrange("p c -> c p"))
                        nc.vector.tensor_scalar(
                            vs_rhs[:1, hh, half * 128:(half + 1) * 128],
                            vrow[:], 0.25, None, op0=ALU.mult)

                # ---- pass 2: q, attention, GCN, epilogue
                _projtags[0] = ["psb4", "psb5", "psb6", "psb7"]
                pair_of = {}
                for pi, pc in enumerate(meta["pairs"]):
                    for j2, c in enumerate(pc):
                        pair_of[c] = (pi, j2)
                for t in range(NT):
                    ht = htp.tile([128, 4, 128], BF16, tag="ht")
                    nc.sync.dma_start_transpose(ht[:], Ht[t][:])
                    ssq = smp.tile([128, H], F32, tag="ssq")
                    qhat = stp.tile([128, 1024], BF16, tag="qhat", bufs=3)
                    qchunks = []
                    for nb in range(2):
                        qp = proj_tile()
                        qchunks.append(qp)
                        for k in range(4):
                            nc.tensor.matmul(
                                qp[:], lhsT=ht[:, k, :],
                                rhs=wq_t[:, k * 1024 + nb * 512:
                                            k * 1024 + (nb + 1) * 512],
                                start=(k == 0), stop=(k == 3) and bq_ is None)
                        if bq_ is not None:
                            nc.tensor.matmul(
                                qp[:], lhsT=onesrow[:],
                                rhs=bq_[li][:, nb * 512:(nb + 1) * 512],
                                start=False, stop=True)
                        for h2 in range(2):
                            hh = nb * 2 + h2
                            sq = scp.tile([128, HC], BF16, tag="sq")
                            nc.scalar.activation(
                                sq[:], qp[:, h2 * HC:(h2 + 1) * HC],
                                AF.Square, accum_out=ssq[:, hh:hh + 1])
                    nrmq = smp.tile([128, H], F32, tag="nrmq")
                    nc.scalar.activation(nrmq[:], ssq[:], AF.Sqrt,
                                         scale=16.0, bias=eps16[:, :1])
                    rsq = smp.tile([128, H], F32, tag="rsq")
                    nc.vector.reciprocal(rsq[:], nrmq[:])
                    for hh in range(H):
                        eng = nc.vector if hh % 2 == 0 else nc.scalar
                        if hh % 2 == 0:
                            nc.vector.tensor_scalar(
                                qhat[:, hh * HC:(hh + 1) * HC],
                                qchunks[hh // 2][:, (hh % 2) * HC:(hh % 2 + 1) * HC],
                                rsq[:, hh:hh + 1], None, op0=ALU.mult)
                        else:
                            nc.scalar.activation(
                                qhat[:, hh * HC:(hh + 1) * HC],
                                qchunks[hh // 2][:, (hh % 2) * HC:(hh % 2 + 1) * HC],
                                AF.Copy, scale=rsq[:, hh:hh + 1])
                    qT = htp.tile([128, 8, 128], BF16, tag="qT", bufs=6)
                    nc.sync.dma_start_transpose(qT[:], qhat[:])

                    # ---- GCN for chunk t (gathers merged per chunk pair)
                    ca, cb = int(capA[t]), int(capB[t])
                    nk = int(nkt[t])
                    ko0 = meta["chunk_kcol"][t]
                    pi, j2 = pair_of[t]
                    if j2 == 0 and pi >= 1 and pi + 1 < npairs:
                        issue_pair_gathers(pi + 1, 0)
                        issue_pair_gathers(pi + 1, 1)
                    pc = meta["pairs"][pi]
                    aoff = sum(int(capA[c]) // 128 for c in pc[:j2])
                    boff = sum(int(capB[c]) // 128 for c in pc[:j2])
                    GA, GB = pair_tiles[pi][0], pair_tiles[pi][1]
                    Sc = sp_.tile([128, nk * 128], mybir.dt.float8e4, tag="St",
                                  name=f"S_{li}_{t}")
                    nc.sync.dma_start(
                        Sc[:], T["s_all"][:, ko0 * 128:(ko0 + nk) * 128])
                    gcnP = gcn_tile()
                    for j in range(nk):
                        Gj = (GA[:, aoff + j, :] if j < ca // 128
                              else GB[:, boff + j - ca // 128, :])
                        nc.tensor.matmul(gcnP[:],
                                         lhsT=Sc[:, j * 128:(j + 1) * 128],
                                         rhs=Gj,
                                         start=(j == 0), stop=(j == nk - 1))
                    x2 = stp.tile([128, HC], BF16, tag="x2")
                    nc.scalar.activation(x2[:], gcnP[:], AF.Copy,
                                         scale=dinvc[:, t:t + 1])
                    if debug:
                        nc.sync.dma_start(
                            dbg["x2_dbg"][:, t * HC:(t + 1) * HC], x2[:])

                    x1 = stp.tile([128, HC], BF16, tag="x1")
                    for hh in range(H):
                        ndh = nd_tile(f"nd{li}_{t}_{hh}")
                        nc.tensor.matmul(ndh[:], lhsT=onesrow[:],
                                         rhs=vs_rhs[:1, hh, :],
                                         start=True, stop=False)
                        for kk in range(2):
                            nc.tensor.matmul(
                                ndh[:], lhsT=qT[:, 2 * hh + kk, :],
                                rhs=kvs_rhs[hh][kk][:],
                                start=False, stop=(kk == 1))
                        dtmp = smp.tile([128, 1], F32, tag="dtmp")
                        nc.vector.tensor_scalar(dtmp[:], ndh[:, 256:257],
                                                50000.0, None, op0=ALU.add)
                        rden = smp.tile([128, 1], F32, tag="rden")
                        nc.vector.reciprocal(rden[:], dtmp[:])
                        if hh == 0:
                            nc.vector.tensor_scalar(x1[:], ndh[:, 0:256],
                                                    rden[:, 0:1], None,
                                                    op0=ALU.mult)
                        else:
                            nc.vector.scalar_tensor_tensor(
                                x1[:], ndh[:, 0:256], rden[:, 0:1],
                                x1[:], op0=ALU.mult, op1=ALU.add)
                    if debug:
                        nc.sync.dma_start(
                            dbg["x1_dbg"][:, t * HC:(t + 1) * HC], x1[:])
                    # ---- epilogue: h = relu(bn(alpha*cat + (1-alpha)*h)) * mask
                    for half, xh in ((0, x1), (1, x2)):
                        sl = slice(half * HC, (half + 1) * HC)
                        u = stp.tile([128, HC], BF16, tag=f"u{half}")
                        nc.vector.tensor_tensor(u[:], xh[:], Ht[t][:, sl],
                                                op=ALU.add)
                        w = stp.tile([128, HC], BF16, tag=f"w{half}")
                        nc.vector.tensor_tensor(w[:], u[:], bnsc[li + 1][:, sl],
                                                op=ALU.mult)
                        z = stp.tile([128, HC], BF16, tag=f"z{half}")
                        nc.vector.tensor_tensor(z[:], w[:], bnsh[li + 1][:, sl],
                                                op=ALU.add)
                        nc.vector.tensor_scalar(Ht[t][:, sl], z[:], 0.0,
                                                maskc[:, t:t + 1],
                                                op0=ALU.max, op1=ALU.mult)
                if debug and li == 0:
                    for t in range(NT):
                        nc.sync.dma_start(
                            dbg["h1_dbg"][:, t * D2:(t + 1) * D2], Ht[t][:])

            # ---------------- final: out = h @ fc_out_w (+ bias)
            if meta["out_bias"]:
                fob = cp.tile([128, OUT], F32, tag="fcoutb")
                nc.sync.dma_start(fob[:], T["fcoutb"][:])
            for t in range(NT):
                ht = htp.tile([128, 4, 128], BF16, tag="ht")
                nc.sync.dma_start_transpose(ht[:], Ht[t][:])
                op_ = proj_tile()
                for k in range(4):
                    nc.tensor.matmul(op_[:, 0:OUT], lhsT=ht[:, k, :],
                                     rhs=fcoutw[:, k * OUT:(k + 1) * OUT],
                                     start=(k == 0), stop=(k == 3))
                of = stp.tile([128, OUT], F32, tag="of")
                if meta["out_bias"]:
                    nc.vector.tensor_tensor(of[:], op_[:, 0:OUT], fob[:],
                                            op=ALU.add)
                else:
                    nc.vector.tensor_copy(of[:], op_[:, 0:OUT])
                nc.sync.dma_start(out_d[t * 128:(t + 1) * 128, :], of[:])

    nc.compile()
    return nc


# ------------------------------------------------------------------- runner
class _SpmdRunner:
    def __init__(self, nc, n_cores):
        install_neuronx_cc_hook()
        self.nc = nc
        self.n_cores = n_cores
        partition_name = (nc.partition_id_tensor.name
                          if nc.partition_id_tensor else None)
        in_names, out_names, out_avals = [], [], []
        for alloc in nc.m.functions[0].allocations:
            if not isinstance(alloc, mybir.MemoryLocationSet):
                continue
            name = alloc.memorylocations[0].name
            if alloc.kind == "ExternalInput":
                if name != partition_name:
                    in_names.append(name)
            elif alloc.kind == "ExternalOutput":
                out_names.append(name)
                out_avals.append(jax.core.ShapedArray(
                    tuple(alloc.tensor_shape), mybir.dt.np(alloc.dtype)))
        self.in_names, self.out_names, self.out_avals = \
            in_names, out_names, out_avals
        n_params = len(in_names)
        all_in = list(in_names) + list(out_names)
        if partition_name is not None:
            all_in.append(partition_name)

        def _body(*args):
            operands = list(args)
            if partition_name is not None:
                operands.append(bass2jax.partition_id_tensor())
            outs = _bass_exec_p.bind(
                *operands, out_avals=tuple(out_avals),
                in_names=tuple(all_in), out_names=tuple(out_names),
                lowering_input_output_aliases=(),
                sim_require_finite=True, sim_require_nnan=True, nc=nc)
            return tuple(outs)

        devices = jax.devices()[:n_cores]
        self.mesh = Mesh(np.asarray(devices), ("core",))
        in_specs = (PartitionSpec("core"),) * (n_params + len(out_names))
        out_specs = (PartitionSpec("core"),) * len(out_names)
        self.fn = jax.jit(
            shard_map(_body, mesh=self.mesh, in_specs=in_specs,
                      out_specs=out_specs, check_rep=False),
            keep_unused=True)
        self._dev_in = None

    def set_inputs(self, in_maps):
        n = self.n_cores
        concat = [np.concatenate([np.asarray(in_maps[c][nm]) for c in range(n)],
                                 axis=0) for nm in self.in_names]
        for av in self.out_avals:
            concat.append(np.zeros((n * av.shape[0], *av.shape[1:]), av.dtype))
        sh = jax.sharding.NamedSharding(self.mesh, PartitionSpec("core"))
        self._dev_in = [jax.device_put(a, sh) for a in concat]

    def run(self):
        outs = self.fn(*self._dev_in)
        jax.block_until_ready(outs)
        return [{nm: np.asarray(outs[i]).reshape(
                    self.n_cores, *self.out_avals[i].shape)[c]
                 for i, nm in enumerate(self.out_names)}
                for c in range(self.n_cores)]

    def time_ns(self, iters=10, warmup=2):
        for _ in range(warmup):
            jax.block_until_ready(self.fn(*self._dev_in))
        ts = []
        for _ in range(iters):
            t0 = time.perf_counter_ns()
            jax.block_until_ready(self.fn(*self._dev_in))
            ts.append(time.perf_counter_ns() - t0)
        return min(ts), sorted(ts)[len(ts) // 2]


_CACHE = {}


def _get_runner(meta, debug=False):
    key = (tuple(meta["capA"]), tuple(meta["capB"]), meta["qkv_bias"],
           meta["out_bias"], debug)
    if key not in _CACHE:
        nc = _build_nc(meta, debug=debug)
        _CACHE[key] = _SpmdRunner(nc, NCORES)
    return _CACHE[key]


def kernel(x, edge_index, batch, fc0_w, fc0_b, wq, bq, wk, bk, wv, bv,
           gcn_w, gcn_b, bn_gamma, bn_beta, bn_mean, bn_var,
           fc_out_w, fc_out_b, _debug=False, _return_runner=False):
    in_maps, meta = _host_prep(
        x, edge_index, fc0_w, fc0_b, wq, bq, wk, bk, wv, bv,
        gcn_w, gcn_b, bn_gamma, bn_beta, bn_mean, bn_var,
        fc_out_w, fc_out_b)
    runner = _get_runner(meta, debug=_debug)
    runner.set_inputs(in_maps)
    results = runner.run()
    out = np.concatenate([results[c]["out"] for c in range(NCORES)], axis=0)
    out = out[:N].astype(np.float32)
    if _return_runner:
        return out, runner, results
    return out



# revision 6
# speedup vs baseline: 2.0488x; 2.0488x over previous
"""DIFFormer (linear attention + GCN) Trainium2 kernel, 8-core SPMD.

Self-contained: only numpy + concourse imports. Hardcoded shapes for
N=50000, E=800000, IN=512, HC=256, H=4, L=2, OUT=64.

v2 structure:
- h state kept TRANSPOSED per tile (feat-major [128, 4, 128]) so
  projections need no per-pass DMA transpose and the BN+relu epilogue
  runs on the Scalar engine with per-partition scale/bias.
- each layer: pass A (xw projection -> AllGather tables early),
  pass B (k,v projections + kvs/ks accumulation; vs comes from an
  hsum row accumulated by the previous epilogue's accum_out),
  AllReduce right after the flush, then pass 2 (q, attention, GCN
  scatter via dma_gather + one-hot fp8 matmuls, transposed epilogue).
"""

import time
import numpy as np

import jax
from jax.sharding import Mesh, PartitionSpec
from jax.experimental.shard_map import shard_map

import concourse.bass as bass
import concourse.bacc as bacc
import concourse.tile as tile
import concourse.mybir as mybir
from concourse import bass2jax, library_config
from concourse.bass2jax import _bass_exec_p, install_neuronx_cc_hook

# ---------------------------------------------------------------- constants
N, E, IN, HC, H, L, OUT = 50000, 800000, 512, 256, 4, 2, 64
D2 = 2 * HC                       # 512
ALPHA, EPS = 0.5, 1e-5
NCORES = 8
NT = 49                           # node tiles per core
NLOC = NT * 128                   # 6272
NP = NCORES * NLOC                # 50176
SPLIT1 = 3200                     # local rows in table1 (tiles 0-24)
SPLIT2 = NLOC - SPLIT1            # 3072 (tiles 25-48)
TB1 = NCORES * SPLIT1             # 25600 rows, int16-safe
TB2 = NCORES * SPLIT2             # 24576 rows, int16-safe
P = 128
BF = np.dtype("bfloat16")
F32 = mybir.dt.float32
BF16 = mybir.dt.bfloat16
I16 = mybir.dt.int16
AF = mybir.ActivationFunctionType
ALU = mybir.AluOpType
PAD_SENTINEL = 300.0              # dst-slot value for padded edge slots


def _roundup(x, m):
    return (x + m - 1) // m * m


def _wrap_idx(arr):
    """int16 index array (len multiple of 16) -> [128, len/16] wrapped layout:
    idx j at partition j%16, col j//16, replicated across 8 Q7 cores."""
    a = arr.reshape(-1, 16).T  # [16, len/16]
    return np.tile(a, (8, 1)).astype(np.int16)


def _host_prep(x, edge_index, fc0_w, fc0_b, wq, bq, wk, bk, wv, bv,
               gcn_w, gcn_b, bn_gamma, bn_beta, bn_mean, bn_var,
               fc_out_w, fc_out_b):
    """Build all per-core device input arrays + compile-time metadata."""
    meta = {}

    # ---- edges: append self-loops, degree norm, sort by dst
    src = np.asarray(edge_index[0], dtype=np.int64)
    dst = np.asarray(edge_index[1], dtype=np.int64)
    src_all = np.concatenate([src, np.arange(N, dtype=np.int64)])
    dst_all = np.concatenate([dst, np.arange(N, dtype=np.int64)])
    deg = np.bincount(dst_all, minlength=NP).astype(np.float64)
    dinv = 1.0 / np.sqrt(np.maximum(deg, 1.0))
    dinv = dinv.astype(np.float32)          # [NP]; pad nodes -> 1.0 (deg 0)

    order = np.argsort(dst_all, kind="stable")
    s_s, s_d = src_all[order], dst_all[order]

    # per (core, chunk) edge lists split into src halves
    nchunks = NT
    cnt = np.zeros((NCORES, nchunks, 2), dtype=np.int64)
    bounds = np.searchsorted(s_d, np.arange(0, NP + 1, 128))
    lists = [[None] * nchunks for _ in range(NCORES)]
    for r in range(NCORES):
        for c in range(nchunks):
            g = r * NT + c
            lo, hi = bounds[g], bounds[g + 1]
            es, ed = s_s[lo:hi], s_d[lo:hi]
            rsrc = es // NLOC
            jloc = es % NLOC
            a_mask = jloc < SPLIT1
            ea = (rsrc[a_mask] * SPLIT1 + jloc[a_mask]).astype(np.int64)
            da = ed[a_mask]
            eb = (rsrc[~a_mask] * SPLIT2 + (jloc[~a_mask] - SPLIT1)).astype(np.int64)
            db = ed[~a_mask]
            lists[r][c] = (ea, da, eb, db)
            cnt[r, c, 0] = len(ea)
            cnt[r, c, 1] = len(eb)

    capA = np.maximum(128, _roundup(cnt[:, :, 0].max(axis=0), 128))  # [nchunks]
    capB = np.maximum(128, _roundup(cnt[:, :, 1].max(axis=0), 128))
    nkt = (capA + capB) // 128                                       # [nchunks]
    meta["capA"], meta["capB"], meta["nkt"] = capA, capB, nkt
    meta["idx_cols"] = int((capA.sum() + capB.sum()) // 16)
    meta["nkt_tot"] = int(nkt.sum())

    # chunk pairs: gathers merged per pair (one A-gather + one B-gather)
    pairs = [list(range(g, min(g + 2, nchunks))) for g in range(0, nchunks, 2)]
    meta["pairs"] = pairs
    idx_all = np.zeros((NCORES, 128, meta["idx_cols"]), dtype=np.int16)
    dst_cols = np.full((NCORES, 128, meta["nkt_tot"]), PAD_SENTINEL,
                       dtype=np.float32)
    icol = 0
    kcol = 0
    meta["pair_icolA"] = {}
    meta["pair_icolB"] = {}
    meta["chunk_kcol"] = [0] * nchunks
    for pi, pc in enumerate(pairs):
        pca = int(sum(capA[c] for c in pc))
        pcb = int(sum(capB[c] for c in pc))
        meta["pair_icolA"][pi] = icol
        meta["pair_icolB"][pi] = icol + pca // 16
        for r in range(NCORES):
            ia = np.zeros(pca, dtype=np.int16)
            ib = np.zeros(pcb, dtype=np.int16)
            oa = ob = 0
            for c in pc:
                ea, da, eb, db = lists[r][c]
                ia[oa:oa + len(ea)] = ea
                ib[ob:ob + len(eb)] = eb
                oa += int(capA[c])
                ob += int(capB[c])
            idx_all[r][:, icol:icol + pca // 16] = _wrap_idx(ia)
            idx_all[r][:, icol + pca // 16:icol + (pca + pcb) // 16] = \
                _wrap_idx(ib)
        icol += (pca + pcb) // 16
        for c in pc:
            meta["chunk_kcol"][c] = kcol
            ca, cb = int(capA[c]), int(capB[c])
            for r in range(NCORES):
                ea, da, eb, db = lists[r][c]
                dloc = np.full(ca + cb, PAD_SENTINEL, dtype=np.float32)
                dloc[:len(da)] = (da - (r * NLOC + c * 128)).astype(np.float32)
                dloc[ca:ca + len(db)] = \
                    (db - (r * NLOC + c * 128)).astype(np.float32)
                dst_cols[r][:, kcol:kcol + (ca + cb) // 128] = \
                    dloc.reshape(-1, 128).T
            kcol += (ca + cb) // 128
    # host-built one-hot selection tiles: S[p, j, x] = (dst_cols[p,j] == x)
    s_hosts = []
    xr = np.arange(128, dtype=np.float32)
    for r in range(NCORES):
        sh_ = (dst_cols[r][:, :, None] == xr).astype(np.float32)
        sh_ = sh_.astype(np.dtype("float8_e4m3fn"))
        s_hosts.append(np.ascontiguousarray(sh_.reshape(128, -1)))

    # ---- per-core node data
    xpad = np.zeros((NP, IN), dtype=np.float32)
    xpad[:N] = np.asarray(x, dtype=np.float32)
    mask = np.zeros((NP,), dtype=np.float32)
    mask[:N] = 1.0
    dinv_m = dinv * mask

    per_core = []
    for r in range(NCORES):
        sl = slice(r * NLOC, (r + 1) * NLOC)
        xs = xpad[sl]                                    # [6272, 512]
        # XT tiles layout [128, NT, 4, 128]: [p,t,k,j] = x[t*128+j, k*128+p]
        xt = np.ascontiguousarray(
            xs.reshape(NT, 128, 4, 128).transpose(3, 0, 2, 1)).astype(BF)
        msl = mask[sl]
        # column mask for the two (possibly) partial tiles 47, 48:
        # cm[p, ti, k, j] = mask(row j of tile 47+ti), replicated over p,k
        cm = np.zeros((2, 128), dtype=np.float32)
        cm[0] = msl[47 * 128:48 * 128]
        cm[1] = msl[48 * 128:49 * 128]
        cmT = np.broadcast_to(cm[None, :, None, :], (128, 2, 4, 128))
        d = {
            "xt": xt.reshape(128, NT * 4 * 128),
            "mask": msl.reshape(NT, 128).T.copy(),               # [128, NT]
            "dinv": dinv_m[sl].reshape(NT, 128).T.copy(),        # [128, NT]
            "cmask": np.ascontiguousarray(
                cmT.reshape(128, 2 * 4 * 128)).astype(BF),
            "idx_all": idx_all[r],
            "s_all": s_hosts[r],
        }
        per_core.append(d)

    # ---- weights / constants (shared across cores)
    def rhs_layout(w):
        # [D2, W] -> [128, 4, W] with [p,k,n] = w[k*128+p, n]
        wv_ = np.asarray(w, dtype=np.float32)
        return np.ascontiguousarray(
            wv_.reshape(4, 128, -1).transpose(1, 0, 2)).astype(BF)

    bn_gamma = np.asarray(bn_gamma, np.float32)
    bn_beta = np.asarray(bn_beta, np.float32)
    bn_mean = np.asarray(bn_mean, np.float32)
    bn_var = np.asarray(bn_var, np.float32)
    fc0_b = np.asarray(fc0_b, np.float32)
    gcn_b = np.asarray(gcn_b, np.float32)

    scale = bn_gamma / np.sqrt(bn_var + EPS)             # [L+1, D2]
    shift = bn_beta - bn_mean * scale
    # BN0 applies to x@W + fc0_b
    sc0, sh0 = scale[0], shift[0] + scale[0] * fc0_b
    bnscale = [sc0.astype(np.float32)]
    bnshift = [sh0.astype(np.float32)]
    for i in range(L):
        sc = ALPHA * scale[i + 1]
        sh = shift[i + 1].copy()
        sh[HC:] += ALPHA * scale[i + 1][HC:] * gcn_b[i]
        bnscale.append(sc.astype(np.float32))
        bnshift.append(sh.astype(np.float32))

    def tcol(v):
        # [512] -> [128, 4] with [p, k] = v[k*128+p]
        return np.ascontiguousarray(
            np.asarray(v, np.float32).reshape(4, 128).T)

    shared = {
        "fc0w": rhs_layout(fc0_w).reshape(128, 4 * D2),
        "fcoutw": rhs_layout(fc_out_w).reshape(128, 4 * OUT),
        "eps": np.full((128, 1), 1e-12, dtype=np.float32),
        "eps16": np.full((128, 1), 16e-12, dtype=np.float32),
        "onesrow": np.ones((1, 128), dtype=np.float32).astype(BF),
        # transposed bn scale/shift columns, [128, (L+1)*4] each
        "bnscT": np.concatenate([tcol(bnscale[j]) for j in range(L + 1)],
                                axis=1),
        "bnshT": np.concatenate([tcol(bnshift[j]) for j in range(L + 1)],
                                axis=1),
    }
    for i in range(L):
        wkv = np.concatenate([np.asarray(wk[i]), np.asarray(wv[i])], axis=1)
        shared[f"wkv{i}"] = rhs_layout(wkv).reshape(128, 4 * 2048)
        shared[f"wq{i}"] = rhs_layout(wq[i]).reshape(128, 4 * 1024)
        shared[f"gcnw{i}"] = rhs_layout(gcn_w[i]).reshape(128, 4 * HC)

    meta["qkv_bias"] = bool(np.any(np.asarray(bq)) or np.any(np.asarray(bk))
                            or np.any(np.asarray(bv)))
    if meta["qkv_bias"]:
        for i in range(L):
            shared[f"bkv{i}"] = np.concatenate(
                [np.asarray(bk[i]), np.asarray(bv[i])]).reshape(1, 2048).astype(BF)
            shared[f"bq{i}"] = np.asarray(bq[i]).reshape(1, 1024).astype(BF)
    meta["out_bias"] = bool(np.any(np.asarray(fc_out_b)))
    if meta["out_bias"]:
        shared["fcoutb"] = np.tile(np.asarray(fc_out_b, np.float32),
                                   (128, 1))

    in_maps = []
    for r in range(NCORES):
        m = dict(per_core[r])
        m.update(shared)
        if meta["qkv_bias"]:
            nvalid = float(min(max(N - r * NLOC, 0), NLOC))
            vsb_bias = np.concatenate(
                [np.asarray(bv[i], np.float32) * nvalid for i in range(L)])
            m["vsbias"] = vsb_bias.reshape(L, 1024)
        in_maps.append(m)
    return in_maps, meta


# ------------------------------------------------------------- program build
def _build_nc(meta, debug=False, single=False):
    nc = bacc.Bacc("TRN2", target_bir_lowering=False, debug=False,
                   num_devices=1 if single else NCORES, num_swdge_queues=4)

    # ---- external inputs
    T = {}
    T["xt"] = nc.dram_tensor("xt", [128, NT * 4 * 128], BF16, kind="ExternalInput")
    T["mask"] = nc.dram_tensor("mask", [128, NT], F32, kind="ExternalInput")
    T["dinv"] = nc.dram_tensor("dinv", [128, NT], F32, kind="ExternalInput")
    T["cmask"] = nc.dram_tensor("cmask", [128, 2 * 4 * 128], BF16,
                                kind="ExternalInput")
    T["idx_all"] = nc.dram_tensor("idx_all", [128, meta["idx_cols"]], I16,
                                  kind="ExternalInput")
    T["s_all"] = nc.dram_tensor("s_all", [128, meta["nkt_tot"] * 128],
                                mybir.dt.float8e4, kind="ExternalInput")
    T["fc0w"] = nc.dram_tensor("fc0w", [128, 4 * D2], BF16, kind="ExternalInput")
    T["fcoutw"] = nc.dram_tensor("fcoutw", [128, 4 * OUT], BF16,
                                 kind="ExternalInput")
    T["eps"] = nc.dram_tensor("eps", [128, 1], F32, kind="ExternalInput")
    T["eps16"] = nc.dram_tensor("eps16", [128, 1], F32, kind="ExternalInput")
    T["onesrow"] = nc.dram_tensor("onesrow", [1, 128], BF16, kind="ExternalInput")
    T["bnscT"] = nc.dram_tensor("bnscT", [128, (L + 1) * 4], F32,
                                kind="ExternalInput")
    T["bnshT"] = nc.dram_tensor("bnshT", [128, (L + 1) * 4], F32,
                                kind="ExternalInput")
    for i in range(L):
        T[f"wkv{i}"] = nc.dram_tensor(f"wkv{i}", [128, 4 * 2048], BF16,
                                      kind="ExternalInput")
        T[f"wq{i}"] = nc.dram_tensor(f"wq{i}", [128, 4 * 1024], BF16,
                                     kind="ExternalInput")
        T[f"gcnw{i}"] = nc.dram_tensor(f"gcnw{i}", [128, 4 * HC], BF16,
                                       kind="ExternalInput")
        if meta["qkv_bias"]:
            T[f"bkv{i}"] = nc.dram_tensor(f"bkv{i}", [1, 2048], BF16,
                                          kind="ExternalInput")
            T[f"bq{i}"] = nc.dram_tensor(f"bq{i}", [1, 1024], BF16,
                                         kind="ExternalInput")
    if meta["qkv_bias"]:
        T["vsbias"] = nc.dram_tensor("vsbias", [L, 1024], F32,
                                     kind="ExternalInput")
    if meta["out_bias"]:
        T["fcoutb"] = nc.dram_tensor("fcoutb", [128, OUT], F32,
                                     kind="ExternalInput")

    out_d = nc.dram_tensor("out", [NLOC, OUT], F32, kind="ExternalOutput")

    # ---- internal DRAM
    xw_in1 = nc.dram_tensor("xw_in1", [SPLIT1, HC], BF16, kind="Internal")
    xw_in2 = nc.dram_tensor("xw_in2", [SPLIT2, HC], BF16, kind="Internal")
    xw_tbl1 = nc.dram_tensor("xw_tbl1", [TB1, HC], BF16, kind="Internal",
                             addr_space="Shared")
    xw_tbl2 = nc.dram_tensor("xw_tbl2", [TB2, HC], BF16, kind="Internal",
                             addr_space="Shared")
    CCK = 4 * 128 * 514           # kvs region floats
    cc_in = nc.dram_tensor("cc_in", [CCK + 1024], F32, kind="Internal")
    cc_out = nc.dram_tensor("cc_out", [CCK + 1024], F32, kind="Internal",
                            addr_space="Shared")
    dbg = {}
    if debug:
        for nm, shp in [("h0_dbg", [128, NT * D2]), ("h1_dbg", [128, NT * D2]),
                        ("x1_dbg", [128, NT * HC]), ("x2_dbg", [128, NT * HC]),
                        ("cc_dbg", [CCK + 1024])]:
            dbg[nm] = nc.dram_tensor(nm, shp, F32 if nm == "cc_dbg" else BF16,
                                     kind="ExternalOutput")

    capA, capB, nkt = meta["capA"], meta["capB"], meta["nkt"]

    with tile.TileContext(nc) as tc:
        with tc.tile_pool(name="const", bufs=1) as cp, \
             tc.tile_pool(name="stage", bufs=3) as stp, \
             tc.tile_pool(name="tpose", bufs=3) as tp, \
             tc.tile_pool(name="scratch", bufs=4) as scp, \
             tc.tile_pool(name="small", bufs=8) as smp, \
             tc.tile_pool(name="gpool", bufs=2) as gp, \
             tc.tile_pool(name="spool", bufs=2) as sp_, \
             tc.tile_pool(name="ps", bufs=1, space="PSUM") as ps:

            nc.gpsimd.load_library(library_config.mlp)

            _cnt = [0, 0, 0]
            _projtags = [["psb6", "psb7"]]

            def proj_tile(shape=None):
                tags = _projtags[0]
                t = ps.tile(shape or [128, 512], F32, space="PSUM",
                            tag=tags[_cnt[0] % len(tags)],
                            name=f"proj{_cnt[0]}")
                _cnt[0] += 1
                return t

            def gcn_tile():
                t = ps.tile([128, HC], F32, space="PSUM",
                            tag=f"psb{2 + _cnt[1] % 2}",
                            name=f"gcn{_cnt[1]}")
                _cnt[1] += 1
                return t

            def nd_tile(name):
                t = ps.tile([128, 257], F32, space="PSUM",
                            tag=f"psb{_cnt[2] % 2}", name=name)
                _cnt[2] += 1
                return t

            # ---- load constants
            def cload(name, shape, dtype):
                t = cp.tile(shape, dtype, tag=name)
                nc.sync.dma_start(t[:], T[name][:])
                return t

            eps = cload("eps", [128, 1], F32)
            eps16 = cload("eps16", [128, 1], F32)
            onesrow = cload("onesrow", [1, 128], BF16)
            maskc = cload("mask", [128, NT], F32)
            dinvc = cload("dinv", [128, NT], F32)
            cmaskT = cp.tile([128, 2, 4, 128], BF16, tag="cmask")
            nc.sync.dma_start(cmaskT[:], T["cmask"][:].rearrange(
                "p (a k j) -> p a k j", a=2, k=4))
            fc0w = cload("fc0w", [128, 4 * D2], BF16)
            fcoutw = cload("fcoutw", [128, 4 * OUT], BF16)
            bnscT = cload("bnscT", [128, (L + 1) * 4], F32)
            bnshT = cload("bnshT", [128, (L + 1) * 4], F32)
            gcnw = [cload(f"gcnw{i}", [128, 4 * HC], BF16) for i in range(L)]
            bkv = bq_ = vsbias = None
            if meta["qkv_bias"]:
                bkv = [cload(f"bkv{i}", [1, 2048], BF16) for i in range(L)]
                bq_ = [cload(f"bq{i}", [1, 1024], BF16) for i in range(L)]
                vsbias = cload("vsbias", [L, 1024], F32)
            maskbf = cp.tile([128, NT], BF16, tag="maskbf")
            nc.vector.tensor_copy(maskbf[:], maskc[:])

            # persistent transposed h tiles  [128 feat, 4(k), 128 rows]
            Ht = [cp.tile([128, 4, 128], BF16, tag=f"H{t}", name=f"H{t}")
                  for t in range(NT)]
            # hsum rows (masked row-sum of h), one per bn level
            hsumT = [cp.tile([128, 4], F32, tag=f"hsum{j}", name=f"hsum{j}")
                     for j in range(L)]

            def epilogue(t, j, src_from_psum=None, u=None, hacc=None):
                """u (or psum) -> transpose -> (optional +Ht) -> bn+relu
                -> Ht[t]; accumulate masked row sums into hacc."""
                if src_from_psum is not None:
                    u = stp.tile([128, D2], BF16, tag="u0")
                    nc.scalar.activation(u[:], src_from_psum[:], AF.Copy)
                uT = tp.tile([128, 4, 128], BF16, tag="uT")
                nc.sync.dma_start_transpose(uT[:], u[:])
                if j > 0:
                    w = tp.tile([128, 4, 128], BF16, tag="wT")
                    nc.vector.tensor_tensor(w[:], uT[:], Ht[t][:], op=ALU.add)
                else:
                    w = uT
                partial = t >= 47
                hs = None
                if hacc is not None and not partial:
                    hs = smp.tile([128, 4], F32, tag="hsa")
                for k in range(4):
                    nc.scalar.activation(
                        Ht[t][:, k, :], w[:, k, :], AF.Relu,
                        scale=bnscT[:, 4 * j + k:4 * j + k + 1],
                        bias=bnshT[:, 4 * j + k:4 * j + k + 1],
                        accum_out=hs[:, k:k + 1] if hs is not None else None)
                if partial:
                    # zero pad rows (they sit in tiles 47/48 only), then
                    # re-accumulate the masked row sums
                    nc.vector.tensor_tensor(Ht[t][:], Ht[t][:],
                                            cmaskT[:, t - 47, :, :],
                                            op=ALU.mult)
                    if hacc is not None:
                        hs = smp.tile([128, 4], F32, tag="hsa")
                        dump = scp.tile([128, 128], BF16, tag="hdump")
                        for k in range(4):
                            nc.scalar.activation(dump[:], Ht[t][:, k, :],
                                                 AF.Copy,
                                                 accum_out=hs[:, k:k + 1])
                if hacc is not None:
                    if t == 0:
                        nc.vector.tensor_copy(hacc[:], hs[:])
                    else:
                        nc.vector.tensor_tensor(hacc[:], hacc[:], hs[:],
                                                op=ALU.add)

            # ---------------- phase 0: h0 = relu(bn0(x @ fc0_w))
            _projtags[0] = ["psb6", "psb7"]
            for t in range(NT):
                xt_t = stp.tile([128, 4, 128], BF16, tag="xt")
                nc.sync.dma_start(
                    xt_t[:], T["xt"][:, t * 512:(t + 1) * 512].rearrange(
                        "p (k j) -> p k j", k=4))
                h0p = proj_tile()
                for k in range(4):
                    nc.tensor.matmul(h0p[:], lhsT=xt_t[:, k, :],
                                     rhs=fc0w[:, k * D2:(k + 1) * D2],
                                     start=(k == 0), stop=(k == 3))
                epilogue(t, 0, src_from_psum=h0p, hacc=hsumT[0])
            if debug:
                for t in range(NT):
                    nc.sync.dma_start(
                        dbg["h0_dbg"][:, t * D2:(t + 1) * D2],
                        Ht[t][:].rearrange("p k j -> p (k j)"))

            # ---------------- layers
            npairs = len(meta["pairs"])
            for li in range(L):
                wkv_t = cp.tile([128, 4, 2048], BF16, tag="wkv",
                                name=f"wkv_l{li}")
                nc.sync.dma_start(wkv_t[:], T[f"wkv{li}"][:].rearrange(
                    "p (k n) -> p k n", k=4))
                wq_t = cp.tile([128, 4 * 1024], BF16, tag="wq",
                               name=f"wq_l{li}")
                nc.sync.dma_start(wq_t[:], T[f"wq{li}"][:])

                # ---- pass A: xw projection -> tables (AllGather early)
                _projtags[0] = ["psb4", "psb5"]
                for t in range(NT):
                    xwp = proj_tile([128, HC])
                    for k in range(4):
                        nc.tensor.matmul(
                            xwp[:], lhsT=Ht[t][:, k, :],
                            rhs=gcnw[li][:, k * HC:(k + 1) * HC],
                            start=(k == 0), stop=(k == 3))
                    xws = stp.tile([128, HC], BF16, tag="xws")
                    nc.scalar.activation(xws[:], xwp[:], AF.Copy,
                                         scale=dinvc[:, t:t + 1])
                    if t < 25:
                        nc.sync.dma_start(
                            xw_in1[t * 128:(t + 1) * 128, :], xws[:])
                    else:
                        nc.sync.dma_start(
                            xw_in2[(t - 25) * 128:(t - 24) * 128, :], xws[:])
                    if t == 24:
                        if single:
                            for _rr in range(NCORES):
                                nc.sync.dma_start(
                                    xw_tbl1[_rr * SPLIT1:(_rr + 1) * SPLIT1, :],
                                    xw_in1[:])
                        else:
                            nc.gpsimd.collective_compute(
                                "AllGather", ALU.bypass,
                                replica_groups=[list(range(NCORES))],
                                ins=[xw_in1[:]], outs=[xw_tbl1[:]])
                if single:
                    for _rr in range(NCORES):
                        nc.sync.dma_start(
                            xw_tbl2[_rr * SPLIT2:(_rr + 1) * SPLIT2, :],
                            xw_in2[:])
                else:
                    nc.gpsimd.collective_compute(
                        "AllGather", ALU.bypass,
                        replica_groups=[list(range(NCORES))],
                        ins=[xw_in2[:]], outs=[xw_tbl2[:]])

                # ---- gather machinery
                pair_tiles = {}

                def issue_pair_gathers(pi2):
                    pc2 = meta["pairs"][pi2]
                    pca2 = int(sum(capA[c] for c in pc2))
                    pcb2 = int(sum(capB[c] for c in pc2))
                    ioA2 = meta["pair_icolA"][pi2]
                    idxp2 = sp_.tile([128, (pca2 + pcb2) // 16], I16,
                                     tag="idxp", bufs=3,
                                     name=f"idxp_{li}_{pi2}")
                    nc.sync.dma_start(
                        idxp2[:],
                        T["idx_all"][:, ioA2:ioA2 + (pca2 + pcb2) // 16])
                    GA2 = gp.tile([128, pca2 // 128, HC], BF16, tag="GA",
                                  name=f"GA_{li}_{pi2}")
                    nc.gpsimd.dma_gather(
                        GA2[:], xw_tbl1[:], idxp2[:, 0:pca2 // 16],
                        pca2, pca2, HC, single_packet=False,
                        queue_num=(2 * pi2) % 4)
                    GB2 = gp.tile([128, pcb2 // 128, HC], BF16, tag="GB",
                                  name=f"GB_{li}_{pi2}")
                    nc.gpsimd.dma_gather(
                        GB2[:], xw_tbl2[:],
                        idxp2[:, pca2 // 16:(pca2 + pcb2) // 16],
                        pcb2, pcb2, HC, single_packet=False,
                        queue_num=(2 * pi2 + 1) % 4)
                    pair_tiles[pi2] = (GA2, GB2)

                issue_pair_gathers(0)
                issue_pair_gathers(1)

                # ---- pass B: k,v projections; kvs/ks accumulation
                kvsP = [ps.tile([128, 512], F32, space="PSUM",
                                tag=f"psb{h}", name=f"kvs{li}_{h}")
                        for h in range(H)]
                ksP = ps.tile([128, 8], F32, space="PSUM", tag="psb4",
                              name=f"ks{li}")
                _projtags[0] = ["psb5", "psb6", "psb7"]

                prev_acc = [None]

                def _emit_acc(kh_, vb_, t_):
                    for hh in range(H):
                        for half in range(2):
                            lhs = kh_[:, hh * HC + half * 128:
                                      hh * HC + half * 128 + 128]
                            nc.tensor.matmul(
                                kvsP[hh][:, half * HC:(half + 1) * HC],
                                lhsT=lhs, rhs=vb_[:, hh * HC:(hh + 1) * HC],
                                start=(t_ == 0), stop=(t_ == NT - 1))
                            nc.tensor.matmul(
                                ksP[:, 2 * hh + half:2 * hh + half + 1],
                                lhsT=lhs, rhs=maskbf[:, t_:t_ + 1],
                                start=(t_ == 0), stop=(t_ == NT - 1))

                for t in range(NT):
                    khat = stp.tile([128, 1024], BF16, tag="khat", bufs=3)
                    vsb = stp.tile([128, 1024], BF16, tag="vsb", bufs=3)
                    for nb in range(4):
                        kvp = proj_tile()
                        for k in range(4):
                            nc.tensor.matmul(
                                kvp[:], lhsT=Ht[t][:, k, :],
                                rhs=wkv_t[:, k, nb * 512:(nb + 1) * 512],
                                start=(k == 0), stop=(k == 3) and bkv is None)
                        if bkv is not None:
                            nc.tensor.matmul(
                                kvp[:], lhsT=onesrow[:],
                                rhs=bkv[li][:, nb * 512:(nb + 1) * 512],
                                start=False, stop=True)
                        if nb < 2:
                            ssk = smp.tile([128, 2], F32, tag="ssk")
                            for h2 in range(2):
                                sq = scp.tile([128, HC], BF16, tag="sq")
                                nc.scalar.activation(
                                    sq[:], kvp[:, h2 * HC:(h2 + 1) * HC],
                                    AF.Square, accum_out=ssk[:, h2:h2 + 1])
                            nrm = smp.tile([128, 2], F32, tag="nrmk")
                            nc.scalar.activation(nrm[:], ssk[:], AF.Sqrt,
                                                 bias=eps[:, :1])
                            rskm = smp.tile([128, 2], F32, tag="rskm")
                            nc.vector.reciprocal(rskm[:], nrm[:])
                            nc.vector.tensor_scalar(rskm[:], rskm[:],
                                                    maskc[:, t:t + 1], None,
                                                    op0=ALU.mult)
                            for h2 in range(2):
                                hh = nb * 2 + h2
                                if h2 == 0:
                                    nc.vector.tensor_scalar(
                                        khat[:, hh * HC:(hh + 1) * HC],
                                        kvp[:, h2 * HC:(h2 + 1) * HC],
                                        rskm[:, h2:h2 + 1], None,
                                        op0=ALU.mult)
                                else:
                                    nc.scalar.activation(
                                        khat[:, hh * HC:(hh + 1) * HC],
                                        kvp[:, h2 * HC:(h2 + 1) * HC],
                                        AF.Copy, scale=rskm[:, h2:h2 + 1])
                        else:
                            sl = slice((nb - 2) * 512, (nb - 1) * 512)
                            if nb == 2:
                                nc.vector.tensor_copy(vsb[:, sl], kvp[:])
                            else:
                                nc.scalar.activation(vsb[:, sl], kvp[:],
                                                     AF.Copy)
                    if prev_acc[0] is not None:
                        _emit_acc(*prev_acc[0])
                    prev_acc[0] = (khat, vsb, t)
                _emit_acc(*prev_acc[0])

                # ---- vs = hsum @ Wv  (+ nvalid * bv)
                hsbf = smp.tile([128, 4], BF16, tag="hsbf")
                nc.vector.tensor_copy(hsbf[:], hsumT[li][:])
                vsP = [ps.tile([1, 512], F32, space="PSUM", tag=f"psb{5 + i}",
                               name=f"vs{li}_{i}") for i in range(2)]
                for i in range(2):
                    for k in range(4):
                        nc.tensor.matmul(
                            vsP[i][:], lhsT=hsbf[:, k:k + 1],
                            rhs=wkv_t[:, k, 1024 + i * 512:1024 + (i + 1) * 512],
                            start=(k == 0), stop=(k == 3))

                # ---- flush kvs/ks/vs to cc_in, AllReduce
                cc_kvs = cc_in[:CCK].rearrange("(h p c) -> h p c", h=4, p=128)
                cc_vs = cc_in[CCK:].rearrange("(o c) -> o c", o=1)
                cco_kvs = cc_out[:CCK].rearrange("(h p c) -> h p c", h=4, p=128)
                cco_vs = cc_out[CCK:].rearrange("(o c) -> o c", o=1)
                for hh in range(H):
                    stg = stp.tile([128, 514], F32, tag="ccstage", bufs=2)
                    for half in range(2):
                        nc.vector.tensor_copy(
                            stg[:, half * 257:half * 257 + 256],
                            kvsP[hh][:, half * HC:(half + 1) * HC])
                        nc.vector.tensor_copy(
                            stg[:, half * 257 + 256:half * 257 + 257],
                            ksP[:, 2 * hh + half:2 * hh + half + 1])
                    nc.sync.dma_start(cc_kvs[hh], stg[:])
                vstg = cp.tile([1, 1024], F32, tag="vstage",
                               name=f"vstage{li}")
                for i in range(2):
                    nc.vector.tensor_copy(vstg[:, i * 512:(i + 1) * 512],
                                          vsP[i][:])
                if vsbias is not None:
                    nc.vector.tensor_tensor(vstg[:], vstg[:],
                                            vsbias[li:li + 1, :], op=ALU.add)
                nc.sync.dma_start(cc_vs, vstg[:])
                if single:
                    nc.sync.dma_start(cc_out[:], cc_in[:])
                else:
                    nc.gpsimd.collective_compute(
                        "AllReduce", ALU.add,
                        replica_groups=[list(range(NCORES))],
                        ins=[cc_in[:]], outs=[cc_out[:]])
                if debug:
                    nc.sync.dma_start(dbg["cc_dbg"][:], cc_out[:])

                # ---- load reduced stats: rhs tiles (0.25-scaled kvs, ks col)
                kvs_rhs = []
                for hh in range(H):
                    row = []
                    for half in range(2):
                        f32t = stp.tile([128, 257], F32, tag="ccload",
                                        bufs=2)
                        nc.sync.dma_start(
                            f32t[:], cco_kvs[hh][:, half * 257:(half + 1) * 257])
                        bft = cp.tile([128, 257], BF16, tag=f"kvsr{hh}_{half}",
                                      name=f"kvsr{li}_{hh}_{half}")
                        nc.vector.tensor_scalar(bft[:, 0:256], f32t[:, 0:256],
                                                0.25, None, op0=ALU.mult)
                        nc.vector.tensor_copy(bft[:, 256:257], f32t[:, 256:257])
                        row.append(bft)
                    kvs_rhs.append(row)
                vs_rhs = cp.tile([1, H, 257], BF16, tag="vsr", name=f"vsr{li}")
                nc.vector.memset(vs_rhs[:], 0)
                vrow = cp.tile([1, 1024], F32, tag="vsload",
                               name=f"vsload{li}")
                nc.sync.dma_start(vrow[:], cco_vs[:])
                for hh in range(H):
                    nc.vector.tensor_scalar(
                        vs_rhs[:1, hh, 0:256],
                        vrow[:, hh * 256:(hh + 1) * 256], 0.25, None,
                        op0=ALU.mult)

                # ---- pass 2: q, attention, GCN, epilogue
                _projtags[0] = ["psb4", "psb5", "psb6", "psb7"]
                pair_of = {}
                for pi, pc in enumerate(meta["pairs"]):
                    for j2, c in enumerate(pc):
                        pair_of[c] = (pi, j2)
                for t in range(NT):
                    ssq = smp.tile([128, H], F32, tag="ssq")
                    qhat = stp.tile([128, 1024], BF16, tag="qhat", bufs=3)
                    qchunks = []
                    for nb in range(2):
                        qp = proj_tile()
                        qchunks.append(qp)
                        for k in range(4):
                            nc.tensor.matmul(
                                qp[:], lhsT=Ht[t][:, k, :],
                                rhs=wq_t[:, k * 1024 + nb * 512:
                                            k * 1024 + (nb + 1) * 512],
                                start=(k == 0), stop=(k == 3) and bq_ is None)
                        if bq_ is not None:
                            nc.tensor.matmul(
                                qp[:], lhsT=onesrow[:],
                                rhs=bq_[li][:, nb * 512:(nb + 1) * 512],
                                start=False, stop=True)
                        for h2 in range(2):
                            hh = nb * 2 + h2
                            sq = scp.tile([128, HC], BF16, tag="sq")
                            nc.scalar.activation(
                                sq[:], qp[:, h2 * HC:(h2 + 1) * HC],
                                AF.Square, accum_out=ssq[:, hh:hh + 1])
                    nrmq = smp.tile([128, H], F32, tag="nrmq")
                    nc.scalar.activation(nrmq[:], ssq[:], AF.Sqrt,
                                         scale=16.0, bias=eps16[:, :1])
                    rsq = smp.tile([128, H], F32, tag="rsq")
                    nc.vector.reciprocal(rsq[:], nrmq[:])
                    for hh in range(H):
                        if hh % 2 == 0:
                            nc.vector.tensor_scalar(
                                qhat[:, hh * HC:(hh + 1) * HC],
                                qchunks[hh // 2][:, (hh % 2) * HC:(hh % 2 + 1) * HC],
                                rsq[:, hh:hh + 1], None, op0=ALU.mult)
                        else:
                            nc.scalar.activation(
                                qhat[:, hh * HC:(hh + 1) * HC],
                                qchunks[hh // 2][:, (hh % 2) * HC:(hh % 2 + 1) * HC],
                                AF.Copy, scale=rsq[:, hh:hh + 1])
                    qT = tp.tile([128, 8, 128], BF16, tag="qT", bufs=3)
                    nc.sync.dma_start_transpose(qT[:], qhat[:])

                    u = stp.tile([128, D2], BF16, tag="ucat", bufs=3)

                    # ---- GCN for chunk t
                    ca, cb = int(capA[t]), int(capB[t])
                    nk = int(nkt[t])
                    ko0 = meta["chunk_kcol"][t]
                    pi, j2 = pair_of[t]
                    if j2 == 0 and pi + 2 < npairs:
                        issue_pair_gathers(pi + 2)
                    pc = meta["pairs"][pi]
                    aoff = sum(int(capA[c]) // 128 for c in pc[:j2])
                    boff = sum(int(capB[c]) // 128 for c in pc[:j2])
                    GA, GB = pair_tiles[pi]
                    Sc = sp_.tile([128, nk * 128], mybir.dt.float8e4, tag="St",
                                  name=f"S_{li}_{t}")
                    nc.sync.dma_start(
                        Sc[:], T["s_all"][:, ko0 * 128:(ko0 + nk) * 128])
                    gcnP = gcn_tile()
                    for j in range(nk):
                        Gj = (GA[:, aoff + j, :] if j < ca // 128
                              else GB[:, boff + j - ca // 128, :])
                        nc.tensor.matmul(gcnP[:],
                                         lhsT=Sc[:, j * 128:(j + 1) * 128],
                                         rhs=Gj,
                                         start=(j == 0), stop=(j == nk - 1))
                    nc.scalar.activation(u[:, HC:], gcnP[:], AF.Copy,
                                         scale=dinvc[:, t:t + 1])
                    if debug:
                        nc.sync.dma_start(
                            dbg["x2_dbg"][:, t * HC:(t + 1) * HC], u[:, HC:])

                    # ---- attention numerator/denominator
                    for hh in range(H):
                        ndh = nd_tile(f"nd{li}_{t}_{hh}")
                        nc.tensor.matmul(ndh[:], lhsT=onesrow[:],
                                         rhs=vs_rhs[:1, hh, :],
                                         start=True, stop=False)
                        for kk in range(2):
                            nc.tensor.matmul(
                                ndh[:], lhsT=qT[:, 2 * hh + kk, :],
                                rhs=kvs_rhs[hh][kk][:],
                                start=False, stop=(kk == 1))
                        dtmp = smp.tile([128, 1], F32, tag="dtmp")
                        nc.vector.tensor_scalar(dtmp[:], ndh[:, 256:257],
                                                50000.0, None, op0=ALU.add)
                        rden = smp.tile([128, 1], F32, tag="rden")
                        nc.vector.reciprocal(rden[:], dtmp[:])
                        if hh == 0:
                            nc.scalar.activation(u[:, 0:HC], ndh[:, 0:256],
                                                 AF.Copy,
                                                 scale=rden[:, 0:1])
                        else:
                            nc.vector.scalar_tensor_tensor(
                                u[:, 0:HC], ndh[:, 0:256], rden[:, 0:1],
                                u[:, 0:HC], op0=ALU.mult, op1=ALU.add)
                    if debug:
                        nc.sync.dma_start(
                            dbg["x1_dbg"][:, t * HC:(t + 1) * HC], u[:, 0:HC])
                    # ---- epilogue (transposed)
                    epilogue(t, li + 1, u=u,
                             hacc=hsumT[li + 1] if li + 1 < L else None)
                if debug and li == 0:
                    for t in range(NT):
                        nc.sync.dma_start(
                            dbg["h1_dbg"][:, t * D2:(t + 1) * D2],
                            Ht[t][:].rearrange("p k j -> p (k j)"))

            # ---------------- final: out = h @ fc_out_w (+ bias)
            _projtags[0] = ["psb6", "psb7"]
            if meta["out_bias"]:
                fob = cp.tile([128, OUT], F32, tag="fcoutb")
                nc.sync.dma_start(fob[:], T["fcoutb"][:])
            for t in range(NT):
                op_ = proj_tile([128, OUT])
                for k in range(4):
                    nc.tensor.matmul(op_[:], lhsT=Ht[t][:, k, :],
                                     rhs=fcoutw[:, k * OUT:(k + 1) * OUT],
                                     start=(k == 0), stop=(k == 3))
                of = stp.tile([128, OUT], F32, tag="of")
                if meta["out_bias"]:
                    nc.vector.tensor_tensor(of[:], op_[:], fob[:],
                                            op=ALU.add)
                else:
                    nc.vector.tensor_copy(of[:], op_[:])
                nc.sync.dma_start(out_d[t * 128:(t + 1) * 128, :], of[:])

    nc.compile()
    return nc


# ------------------------------------------------------------------- runner
class _SpmdRunner:
    def __init__(self, nc, n_cores):
        install_neuronx_cc_hook()
        self.nc = nc
        self.n_cores = n_cores
        partition_name = (nc.partition_id_tensor.name
                          if nc.partition_id_tensor else None)
        in_names, out_names, out_avals = [], [], []
        for alloc in nc.m.functions[0].allocations:
            if not isinstance(alloc, mybir.MemoryLocationSet):
                continue
            name = alloc.memorylocations[0].name
            if alloc.kind == "ExternalInput":
                if name != partition_name:
                    in_names.append(name)
            elif alloc.kind == "ExternalOutput":
                out_names.append(name)
                out_avals.append(jax.core.ShapedArray(
                    tuple(alloc.tensor_shape), mybir.dt.np(alloc.dtype)))
        self.in_names, self.out_names, self.out_avals = \
            in_names, out_names, out_avals
        n_params = len(in_names)
        all_in = list(in_names) + list(out_names)
        if partition_name is not None:
            all_in.append(partition_name)

        def _body(*args):
            operands = list(args)
            if partition_name is not None:
                operands.append(bass2jax.partition_id_tensor())
            outs = _bass_exec_p.bind(
                *operands, out_avals=tuple(out_avals),
                in_names=tuple(all_in), out_names=tuple(out_names),
                lowering_input_output_aliases=(),
                sim_require_finite=True, sim_require_nnan=True, nc=nc)
            return tuple(outs)

        devices = jax.devices()[:n_cores]
        self.mesh = Mesh(np.asarray(devices), ("core",))
        in_specs = (PartitionSpec("core"),) * (n_params + len(out_names))
        out_specs = (PartitionSpec("core"),) * len(out_names)
        self.fn = jax.jit(
            shard_map(_body, mesh=self.mesh, in_specs=in_specs,
                      out_specs=out_specs, check_rep=False),
            keep_unused=True)
        self._dev_in = None

    def set_inputs(self, in_maps):
        n = self.n_cores
        concat = [np.concatenate([np.asarray(in_maps[c][nm]) for c in range(n)],
                                 axis=0) for nm in self.in_names]
        for av in self.out_avals:
            concat.append(np.zeros((n * av.shape[0], *av.shape[1:]), av.dtype))
        sh = jax.sharding.NamedSharding(self.mesh, PartitionSpec("core"))
        self._dev_in = [jax.device_put(a, sh) for a in concat]

    def run(self):
        outs = self.fn(*self._dev_in)
        jax.block_until_ready(outs)
        return [{nm: np.asarray(outs[i]).reshape(
                    self.n_cores, *self.out_avals[i].shape)[c]
                 for i, nm in enumerate(self.out_names)}
                for c in range(self.n_cores)]

    def time_ns(self, iters=10, warmup=2):
        for _ in range(warmup):
            jax.block_until_ready(self.fn(*self._dev_in))
        ts = []
        for _ in range(iters):
            t0 = time.perf_counter_ns()
            jax.block_until_ready(self.fn(*self._dev_in))
            ts.append(time.perf_counter_ns() - t0)
        return min(ts), sorted(ts)[len(ts) // 2]


_CACHE = {}


def _get_runner(meta, debug=False):
    key = (tuple(meta["capA"]), tuple(meta["capB"]), meta["qkv_bias"],
           meta["out_bias"], debug)
    if key not in _CACHE:
        nc = _build_nc(meta, debug=debug)
        _CACHE[key] = _SpmdRunner(nc, NCORES)
    return _CACHE[key]


def kernel(x, edge_index, batch, fc0_w, fc0_b, wq, bq, wk, bk, wv, bv,
           gcn_w, gcn_b, bn_gamma, bn_beta, bn_mean, bn_var,
           fc_out_w, fc_out_b, _debug=False, _return_runner=False):
    in_maps, meta = _host_prep(
        x, edge_index, fc0_w, fc0_b, wq, bq, wk, bk, wv, bv,
        gcn_w, gcn_b, bn_gamma, bn_beta, bn_mean, bn_var,
        fc_out_w, fc_out_b)
    runner = _get_runner(meta, debug=_debug)
    runner.set_inputs(in_maps)
    results = runner.run()
    out = np.concatenate([results[c]["out"] for c in range(NCORES)], axis=0)
    out = out[:N].astype(np.float32)
    if _return_runner:
        return out, runner, results
    return out


# revision 7
# speedup vs baseline: 2.2216x; 1.0843x over previous
"""DIFFormer (linear attention + GCN) Trainium2 kernel, 8-core SPMD.

Self-contained: only numpy + concourse imports. Hardcoded shapes for
N=50000, E=800000, IN=512, HC=256, H=4, L=2, OUT=64.

v2 structure:
- h state kept TRANSPOSED per tile (feat-major [128, 4, 128]) so
  projections need no per-pass DMA transpose and the BN+relu epilogue
  runs on the Scalar engine with per-partition scale/bias.
- each layer: pass A (xw projection -> AllGather tables early),
  pass B (k,v projections + kvs/ks accumulation; vs comes from an
  hsum row accumulated by the previous epilogue's accum_out),
  AllReduce right after the flush, then pass 2 (q, attention, GCN
  scatter via dma_gather + one-hot fp8 matmuls, transposed epilogue).
"""

import time
import numpy as np

import jax
from jax.sharding import Mesh, PartitionSpec
from jax.experimental.shard_map import shard_map

import concourse.bass as bass
import concourse.bacc as bacc
import concourse.tile as tile
import concourse.mybir as mybir
from concourse import bass2jax, library_config
from concourse.bass2jax import _bass_exec_p, install_neuronx_cc_hook

# ---------------------------------------------------------------- constants
N, E, IN, HC, H, L, OUT = 50000, 800000, 512, 256, 4, 2, 64
D2 = 2 * HC                       # 512
ALPHA, EPS = 0.5, 1e-5
NCORES = 8
NT = 49                           # node tiles per core
NLOC = NT * 128                   # 6272
NP = NCORES * NLOC                # 50176
SPLIT1 = 3200                     # local rows in table1 (tiles 0-24)
SPLIT2 = NLOC - SPLIT1            # 3072 (tiles 25-48)
TB1 = NCORES * SPLIT1             # 25600 rows, int16-safe
TB2 = NCORES * SPLIT2             # 24576 rows, int16-safe
P = 128
BF = np.dtype("bfloat16")
F32 = mybir.dt.float32
BF16 = mybir.dt.bfloat16
I16 = mybir.dt.int16
AF = mybir.ActivationFunctionType
ALU = mybir.AluOpType
PAD_SENTINEL = 300.0              # dst-slot value for padded edge slots
GFP8 = True                       # gather tables / GCN rhs in fp8e4
NSLOT = 4                         # gather pair slots in flight (prep-ahead)
GNP = np.dtype("float8_e4m3fn") if GFP8 else np.dtype("bfloat16")


def _roundup(x, m):
    return (x + m - 1) // m * m


def _wrap_idx(arr):
    """int16 index array (len multiple of 16) -> [128, len/16] wrapped layout:
    idx j at partition j%16, col j//16, replicated across 8 Q7 cores."""
    a = arr.reshape(-1, 16).T  # [16, len/16]
    return np.tile(a, (8, 1)).astype(np.int16)


def _host_prep(x, edge_index, fc0_w, fc0_b, wq, bq, wk, bk, wv, bv,
               gcn_w, gcn_b, bn_gamma, bn_beta, bn_mean, bn_var,
               fc_out_w, fc_out_b):
    """Build all per-core device input arrays + compile-time metadata."""
    meta = {}

    # ---- edges: append self-loops, degree norm, sort by dst
    src_all = np.asarray(edge_index[0], dtype=np.int64)
    dst_all = np.asarray(edge_index[1], dtype=np.int64)
    # degree includes the self-loop (handled locally on-device, not gathered)
    deg = (np.bincount(dst_all, minlength=NP)
           + (np.arange(NP) < N)).astype(np.float64)
    dinv = 1.0 / np.sqrt(np.maximum(deg, 1.0))
    dinv = dinv.astype(np.float32)          # [NP]; pad nodes -> 1.0 (deg 0)

    order = np.argsort(dst_all, kind="stable")
    s_s, s_d = src_all[order], dst_all[order]

    # per (core, chunk) edge lists split into src halves
    nchunks = NT
    cnt = np.zeros((NCORES, nchunks, 2), dtype=np.int64)
    bounds = np.searchsorted(s_d, np.arange(0, NP + 1, 128))
    lists = [[None] * nchunks for _ in range(NCORES)]
    for r in range(NCORES):
        for c in range(nchunks):
            g = r * NT + c
            lo, hi = bounds[g], bounds[g + 1]
            es, ed = s_s[lo:hi], s_d[lo:hi]
            rsrc = es // NLOC
            jloc = es % NLOC
            a_mask = jloc < SPLIT1
            ea = (rsrc[a_mask] * SPLIT1 + jloc[a_mask]).astype(np.int64)
            da = ed[a_mask]
            eb = (rsrc[~a_mask] * SPLIT2 + (jloc[~a_mask] - SPLIT1)).astype(np.int64)
            db = ed[~a_mask]
            lists[r][c] = (ea, da, eb, db)
            cnt[r, c, 0] = len(ea)
            cnt[r, c, 1] = len(eb)

    capA = np.maximum(128, _roundup(cnt[:, :, 0].max(axis=0), 128))  # [nchunks]
    capB = np.maximum(128, _roundup(cnt[:, :, 1].max(axis=0), 128))
    nkt = (capA + capB) // 128                                       # [nchunks]
    meta["capA"], meta["capB"], meta["nkt"] = capA, capB, nkt
    meta["idx_cols"] = int((capA.sum() + capB.sum()) // 16)
    meta["nkt_tot"] = int(nkt.sum())

    # chunk pairs: gathers merged per pair (one A-gather + one B-gather)
    pairs = [list(range(g, min(g + 2, nchunks))) for g in range(0, nchunks, 2)]
    meta["pairs"] = pairs
    idx_all = np.zeros((NCORES, 128, meta["idx_cols"]), dtype=np.int16)
    dst_cols = np.full((NCORES, 128, meta["nkt_tot"]), PAD_SENTINEL,
                       dtype=np.float32)
    icol = 0
    kcol = 0
    meta["pair_icolA"] = {}
    meta["pair_icolB"] = {}
    meta["chunk_kcol"] = [0] * nchunks
    for pi, pc in enumerate(pairs):
        pca = int(sum(capA[c] for c in pc))
        pcb = int(sum(capB[c] for c in pc))
        meta["pair_icolA"][pi] = icol
        meta["pair_icolB"][pi] = icol + pca // 16
        for r in range(NCORES):
            ia = np.zeros(pca, dtype=np.int16)
            ib = np.zeros(pcb, dtype=np.int16)
            oa = ob = 0
            for c in pc:
                ea, da, eb, db = lists[r][c]
                ia[oa:oa + len(ea)] = ea
                ib[ob:ob + len(eb)] = eb
                oa += int(capA[c])
                ob += int(capB[c])
            idx_all[r][:, icol:icol + pca // 16] = _wrap_idx(ia)
            idx_all[r][:, icol + pca // 16:icol + (pca + pcb) // 16] = \
                _wrap_idx(ib)
        icol += (pca + pcb) // 16
        for c in pc:
            meta["chunk_kcol"][c] = kcol
            ca, cb = int(capA[c]), int(capB[c])
            for r in range(NCORES):
                ea, da, eb, db = lists[r][c]
                dloc = np.full(ca + cb, PAD_SENTINEL, dtype=np.float32)
                dloc[:len(da)] = (da - (r * NLOC + c * 128)).astype(np.float32)
                dloc[ca:ca + len(db)] = \
                    (db - (r * NLOC + c * 128)).astype(np.float32)
                dst_cols[r][:, kcol:kcol + (ca + cb) // 128] = \
                    dloc.reshape(-1, 128).T
            kcol += (ca + cb) // 128
    # host-built one-hot selection tiles: S[p, j, x] = (dst_cols[p,j] == x)
    s_hosts = []
    xr = np.arange(128, dtype=np.float32)
    for r in range(NCORES):
        sh_ = (dst_cols[r][:, :, None] == xr).astype(np.float32)
        sh_ = sh_.astype(np.dtype("float8_e4m3fn"))
        s_hosts.append(np.ascontiguousarray(sh_.reshape(128, -1)))

    # ---- per-core node data
    xpad = np.zeros((NP, IN), dtype=np.float32)
    xpad[:N] = np.asarray(x, dtype=np.float32)
    mask = np.zeros((NP,), dtype=np.float32)
    mask[:N] = 1.0
    dinv_m = dinv * mask

    per_core = []
    for r in range(NCORES):
        sl = slice(r * NLOC, (r + 1) * NLOC)
        xs = xpad[sl]                                    # [6272, 512]
        # XT tiles layout [128, NT, 4, 128]: [p,t,k,j] = x[t*128+j, k*128+p]
        xt = np.ascontiguousarray(
            xs.reshape(NT, 128, 4, 128).transpose(3, 0, 2, 1)).astype(BF)
        msl = mask[sl]
        # column mask for the two (possibly) partial tiles 47, 48:
        # cm[p, ti, k, j] = mask(row j of tile 47+ti), replicated over p,k
        cm = np.zeros((2, 128), dtype=np.float32)
        cm[0] = msl[47 * 128:48 * 128]
        cm[1] = msl[48 * 128:49 * 128]
        cmT = np.broadcast_to(cm[None, :, None, :], (128, 2, 4, 128))
        d = {
            "xt": xt.reshape(128, NT * 4 * 128),
            "mask": msl.reshape(NT, 128).T.copy(),               # [128, NT]
            "dinv": dinv_m[sl].reshape(NT, 128).T.copy(),        # [128, NT]
            "cmask": np.ascontiguousarray(
                cmT.reshape(128, 2 * 4 * 128)).astype(BF),
            "idx_all": idx_all[r],
            "s_all": s_hosts[r],
        }
        per_core.append(d)

    # ---- weights / constants (shared across cores)
    def rhs_layout(w):
        # [D2, W] -> [128, 4, W] with [p,k,n] = w[k*128+p, n]
        wv_ = np.asarray(w, dtype=np.float32)
        return np.ascontiguousarray(
            wv_.reshape(4, 128, -1).transpose(1, 0, 2)).astype(BF)

    bn_gamma = np.asarray(bn_gamma, np.float32)
    bn_beta = np.asarray(bn_beta, np.float32)
    bn_mean = np.asarray(bn_mean, np.float32)
    bn_var = np.asarray(bn_var, np.float32)
    fc0_b = np.asarray(fc0_b, np.float32)
    gcn_b = np.asarray(gcn_b, np.float32)

    scale = bn_gamma / np.sqrt(bn_var + EPS)             # [L+1, D2]
    shift = bn_beta - bn_mean * scale
    # BN0 applies to x@W + fc0_b
    sc0, sh0 = scale[0], shift[0] + scale[0] * fc0_b
    bnscale = [sc0.astype(np.float32)]
    bnshift = [sh0.astype(np.float32)]
    for i in range(L):
        sc = ALPHA * scale[i + 1]
        sh = shift[i + 1].copy()
        sh[HC:] += ALPHA * scale[i + 1][HC:] * gcn_b[i]
        bnscale.append(sc.astype(np.float32))
        bnshift.append(sh.astype(np.float32))

    def tcol(v):
        # [512] -> [128, 4] with [p, k] = v[k*128+p]
        return np.ascontiguousarray(
            np.asarray(v, np.float32).reshape(4, 128).T)

    shared = {
        "fc0w": rhs_layout(fc0_w).reshape(128, 4 * D2),
        "fcoutw": rhs_layout(fc_out_w).reshape(128, 4 * OUT),
        "eps": np.full((128, 1), 1e-12, dtype=np.float32),
        "eps16": np.full((128, 1), 16e-12, dtype=np.float32),
        "onesrow": np.ones((1, 128), dtype=np.float32).astype(BF),
        # transposed bn scale/shift columns, [128, (L+1)*4] each
        "bnscT": np.concatenate([tcol(bnscale[j]) for j in range(L + 1)],
                                axis=1),
        "bnshT": np.concatenate([tcol(bnshift[j]) for j in range(L + 1)],
                                axis=1),
    }
    for i in range(L):
        wkv = np.concatenate([np.asarray(wk[i]), np.asarray(wv[i])], axis=1)
        shared[f"wkv{i}"] = rhs_layout(wkv).reshape(128, 4 * 2048)
        shared[f"wq{i}"] = rhs_layout(wq[i]).reshape(128, 4 * 1024)
        shared[f"gcnw{i}"] = rhs_layout(gcn_w[i]).reshape(128, 4 * HC)

    meta["qkv_bias"] = bool(np.any(np.asarray(bq)) or np.any(np.asarray(bk))
                            or np.any(np.asarray(bv)))
    if meta["qkv_bias"]:
        for i in range(L):
            shared[f"bkv{i}"] = np.concatenate(
                [np.asarray(bk[i]), np.asarray(bv[i])]).reshape(1, 2048).astype(BF)
            shared[f"bq{i}"] = np.asarray(bq[i]).reshape(1, 1024).astype(BF)
    meta["out_bias"] = bool(np.any(np.asarray(fc_out_b)))
    if meta["out_bias"]:
        shared["fcoutb"] = np.tile(np.asarray(fc_out_b, np.float32),
                                   (128, 1))

    in_maps = []
    for r in range(NCORES):
        m = dict(per_core[r])
        m.update(shared)
        if meta["qkv_bias"]:
            nvalid = float(min(max(N - r * NLOC, 0), NLOC))
            vsb_bias = np.concatenate(
                [np.asarray(bv[i], np.float32) * nvalid for i in range(L)])
            m["vsbias"] = vsb_bias.reshape(L, 1024)
        in_maps.append(m)
    return in_maps, meta


# ------------------------------------------------------------- program build
def _build_nc(meta, debug=False, single=False):
    nc = bacc.Bacc("TRN2", target_bir_lowering=False, debug=False,
                   num_devices=1 if single else NCORES, num_swdge_queues=4)

    # ---- external inputs
    T = {}
    T["xt"] = nc.dram_tensor("xt", [128, NT * 4 * 128], BF16, kind="ExternalInput")
    T["mask"] = nc.dram_tensor("mask", [128, NT], F32, kind="ExternalInput")
    T["dinv"] = nc.dram_tensor("dinv", [128, NT], F32, kind="ExternalInput")
    T["cmask"] = nc.dram_tensor("cmask", [128, 2 * 4 * 128], BF16,
                                kind="ExternalInput")
    T["idx_all"] = nc.dram_tensor("idx_all", [128, meta["idx_cols"]], I16,
                                  kind="ExternalInput")
    T["s_all"] = nc.dram_tensor("s_all", [128, meta["nkt_tot"] * 128],
                                mybir.dt.float8e4, kind="ExternalInput")
    T["fc0w"] = nc.dram_tensor("fc0w", [128, 4 * D2], BF16, kind="ExternalInput")
    T["fcoutw"] = nc.dram_tensor("fcoutw", [128, 4 * OUT], BF16,
                                 kind="ExternalInput")
    T["eps"] = nc.dram_tensor("eps", [128, 1], F32, kind="ExternalInput")
    T["eps16"] = nc.dram_tensor("eps16", [128, 1], F32, kind="ExternalInput")
    T["onesrow"] = nc.dram_tensor("onesrow", [1, 128], BF16, kind="ExternalInput")
    T["bnscT"] = nc.dram_tensor("bnscT", [128, (L + 1) * 4], F32,
                                kind="ExternalInput")
    T["bnshT"] = nc.dram_tensor("bnshT", [128, (L + 1) * 4], F32,
                                kind="ExternalInput")
    for i in range(L):
        T[f"wkv{i}"] = nc.dram_tensor(f"wkv{i}", [128, 4 * 2048], BF16,
                                      kind="ExternalInput")
        T[f"wq{i}"] = nc.dram_tensor(f"wq{i}", [128, 4 * 1024], BF16,
                                     kind="ExternalInput")
        T[f"gcnw{i}"] = nc.dram_tensor(f"gcnw{i}", [128, 4 * HC], BF16,
                                       kind="ExternalInput")
        if meta["qkv_bias"]:
            T[f"bkv{i}"] = nc.dram_tensor(f"bkv{i}", [1, 2048], BF16,
                                          kind="ExternalInput")
            T[f"bq{i}"] = nc.dram_tensor(f"bq{i}", [1, 1024], BF16,
                                         kind="ExternalInput")
    if meta["qkv_bias"]:
        T["vsbias"] = nc.dram_tensor("vsbias", [L, 1024], F32,
                                     kind="ExternalInput")
    if meta["out_bias"]:
        T["fcoutb"] = nc.dram_tensor("fcoutb", [128, OUT], F32,
                                     kind="ExternalInput")

    out_d = nc.dram_tensor("out", [NLOC, OUT], F32, kind="ExternalOutput")

    # ---- internal DRAM
    xw_in1 = nc.dram_tensor("xw_in1", [SPLIT1, HC], BF16, kind="Internal")
    xw_in2 = nc.dram_tensor("xw_in2", [SPLIT2, HC], BF16, kind="Internal")
    xw_tbl1 = nc.dram_tensor("xw_tbl1", [TB1, HC], BF16, kind="Internal",
                             addr_space="Shared")
    xw_tbl2 = nc.dram_tensor("xw_tbl2", [TB2, HC], BF16, kind="Internal",
                             addr_space="Shared")
    CCK = 4 * 128 * 514           # kvs region floats
    cc_in = nc.dram_tensor("cc_in", [CCK + 1024], F32, kind="Internal")
    cc_out = nc.dram_tensor("cc_out", [CCK + 1024], F32, kind="Internal",
                            addr_space="Shared")
    dbg = {}
    if debug:
        for nm, shp in [("h0_dbg", [128, NT * D2]), ("h1_dbg", [128, NT * D2]),
                        ("x1_dbg", [128, NT * HC]), ("x2_dbg", [128, NT * HC]),
                        ("cc_dbg", [CCK + 1024])]:
            dbg[nm] = nc.dram_tensor(nm, shp, F32 if nm == "cc_dbg" else BF16,
                                     kind="ExternalOutput")

    capA, capB, nkt = meta["capA"], meta["capB"], meta["nkt"]

    with tile.TileContext(nc) as tc:
        with tc.tile_pool(name="const", bufs=1) as cp, \
             tc.tile_pool(name="stage", bufs=3) as stp, \
             tc.tile_pool(name="tpose", bufs=3) as tp, \
             tc.tile_pool(name="scratch", bufs=4) as scp, \
             tc.tile_pool(name="small", bufs=8) as smp, \
             tc.tile_pool(name="gpool", bufs=2) as gp, \
             tc.tile_pool(name="spool", bufs=2) as sp_, \
             tc.tile_pool(name="ps", bufs=1, space="PSUM") as ps:

            nc.gpsimd.load_library(library_config.mlp)

            _cnt = [0, 0, 0]
            _projtags = [["psb6", "psb7"]]

            def proj_tile(shape=None):
                tags = _projtags[0]
                t = ps.tile(shape or [128, 512], F32, space="PSUM",
                            tag=tags[_cnt[0] % len(tags)],
                            name=f"proj{_cnt[0]}")
                _cnt[0] += 1
                return t

            def gcn_tile():
                t = ps.tile([128, HC], F32, space="PSUM",
                            tag=f"psb{2 + _cnt[1] % 2}",
                            name=f"gcn{_cnt[1]}")
                _cnt[1] += 1
                return t

            def nd_tile(name):
                t = ps.tile([128, 257], F32, space="PSUM",
                            tag=f"psb{_cnt[2] % 2}", name=name)
                _cnt[2] += 1
                return t

            # ---- load constants
            def cload(name, shape, dtype):
                t = cp.tile(shape, dtype, tag=name)
                nc.sync.dma_start(t[:], T[name][:])
                return t

            eps = cload("eps", [128, 1], F32)
            eps16 = cload("eps16", [128, 1], F32)
            onesrow = cload("onesrow", [1, 128], BF16)
            maskc = cload("mask", [128, NT], F32)
            dinvc = cload("dinv", [128, NT], F32)
            cmaskT = cp.tile([128, 2, 4, 128], BF16, tag="cmask")
            nc.sync.dma_start(cmaskT[:], T["cmask"][:].rearrange(
                "p (a k j) -> p a k j", a=2, k=4))
            fc0w = cload("fc0w", [128, 4 * D2], BF16)
            fcoutw = cload("fcoutw", [128, 4 * OUT], BF16)
            bnscT = cload("bnscT", [128, (L + 1) * 4], F32)
            bnshT = cload("bnshT", [128, (L + 1) * 4], F32)
            gcnw = [cload(f"gcnw{i}", [128, 4 * HC], BF16) for i in range(L)]
            bkv = bq_ = vsbias = None
            if meta["qkv_bias"]:
                bkv = [cload(f"bkv{i}", [1, 2048], BF16) for i in range(L)]
                bq_ = [cload(f"bq{i}", [1, 1024], BF16) for i in range(L)]
                vsbias = cload("vsbias", [L, 1024], F32)
            maskbf = cp.tile([128, NT], BF16, tag="maskbf")
            nc.vector.tensor_copy(maskbf[:], maskc[:])

            # persistent transposed h tiles  [128 feat, 4(k), 128 rows]
            Ht = [cp.tile([128, 4, 128], BF16, tag=f"H{t}", name=f"H{t}")
                  for t in range(NT)]
            # hsum rows (masked row-sum of h), one per bn level
            hsumT = [cp.tile([128, 4], F32, tag=f"hsum{j}", name=f"hsum{j}")
                     for j in range(L)]

            def epilogue(t, j, src_from_psum=None, u=None, hacc=None):
                """u (or psum) -> transpose -> (optional +Ht) -> bn+relu
                -> Ht[t]; accumulate masked row sums into hacc."""
                if src_from_psum is not None:
                    u = stp.tile([128, D2], BF16, tag="u0")
                    nc.scalar.activation(u[:], src_from_psum[:], AF.Copy)
                uT = tp.tile([128, 4, 128], BF16, tag="uT")
                nc.sync.dma_start_transpose(uT[:], u[:])
                if j > 0:
                    w = tp.tile([128, 4, 128], BF16, tag="wT")
                    nc.vector.tensor_tensor(w[:], uT[:], Ht[t][:], op=ALU.add)
                else:
                    w = uT
                partial = t >= 47
                hs = None
                if hacc is not None and not partial:
                    hs = smp.tile([128, 4], F32, tag="hsa")
                for k in range(4):
                    nc.scalar.activation(
                        Ht[t][:, k, :], w[:, k, :], AF.Relu,
                        scale=bnscT[:, 4 * j + k:4 * j + k + 1],
                        bias=bnshT[:, 4 * j + k:4 * j + k + 1],
                        accum_out=hs[:, k:k + 1] if hs is not None else None)
                if partial:
                    # zero pad rows (they sit in tiles 47/48 only), then
                    # re-accumulate the masked row sums
                    nc.vector.tensor_tensor(Ht[t][:], Ht[t][:],
                                            cmaskT[:, t - 47, :, :],
                                            op=ALU.mult)
                    if hacc is not None:
                        hs = smp.tile([128, 4], F32, tag="hsa")
                        dump = scp.tile([128, 128], BF16, tag="hdump")
                        for k in range(4):
                            nc.scalar.activation(dump[:], Ht[t][:, k, :],
                                                 AF.Copy,
                                                 accum_out=hs[:, k:k + 1])
                if hacc is not None:
                    if t == 0:
                        nc.vector.tensor_copy(hacc[:], hs[:])
                    else:
                        nc.vector.tensor_tensor(hacc[:], hacc[:], hs[:],
                                                op=ALU.add)

            # ---------------- phase 0: h0 = relu(bn0(x @ fc0_w))
            _projtags[0] = ["psb6", "psb7"]
            for t in range(NT):
                xt_t = stp.tile([128, 4, 128], BF16, tag="xt")
                nc.sync.dma_start(
                    xt_t[:], T["xt"][:, t * 512:(t + 1) * 512].rearrange(
                        "p (k j) -> p k j", k=4))
                h0p = proj_tile()
                for k in range(4):
                    nc.tensor.matmul(h0p[:], lhsT=xt_t[:, k, :],
                                     rhs=fc0w[:, k * D2:(k + 1) * D2],
                                     start=(k == 0), stop=(k == 3))
                epilogue(t, 0, src_from_psum=h0p, hacc=hsumT[0])
            if debug:
                for t in range(NT):
                    nc.sync.dma_start(
                        dbg["h0_dbg"][:, t * D2:(t + 1) * D2],
                        Ht[t][:].rearrange("p k j -> p (k j)"))

            # ---------------- layers
            npairs = len(meta["pairs"])
            for li in range(L):
                wkv_t = cp.tile([128, 4, 2048], BF16, tag="wkv",
                                name=f"wkv_l{li}")
                nc.sync.dma_start(wkv_t[:], T[f"wkv{li}"][:].rearrange(
                    "p (k n) -> p k n", k=4))
                wq_t = cp.tile([128, 4 * 1024], BF16, tag="wq",
                               name=f"wq_l{li}")
                nc.sync.dma_start(wq_t[:], T[f"wq{li}"][:])

                # ---- pass A: xw projection -> tables (AllGather early)
                _projtags[0] = ["psb4", "psb5"]
                for t in range(NT):
                    xwp = proj_tile([128, HC])
                    for k in range(4):
                        nc.tensor.matmul(
                            xwp[:], lhsT=Ht[t][:, k, :],
                            rhs=gcnw[li][:, k * HC:(k + 1) * HC],
                            start=(k == 0), stop=(k == 3))
                    xws = stp.tile([128, HC], BF16, tag="xws")
                    nc.scalar.activation(xws[:], xwp[:], AF.Copy,
                                         scale=dinvc[:, t:t + 1])
                    if t < 25:
                        nc.sync.dma_start(
                            xw_in1[t * 128:(t + 1) * 128, :], xws[:])
                    else:
                        nc.sync.dma_start(
                            xw_in2[(t - 25) * 128:(t - 24) * 128, :], xws[:])
                    if t == 24:
                        if single:
                            for _rr in range(NCORES):
                                nc.sync.dma_start(
                                    xw_tbl1[_rr * SPLIT1:(_rr + 1) * SPLIT1, :],
                                    xw_in1[:])
                        else:
                            nc.gpsimd.collective_compute(
                                "AllGather", ALU.bypass,
                                replica_groups=[list(range(NCORES))],
                                ins=[xw_in1[:]], outs=[xw_tbl1[:]])
                if single:
                    for _rr in range(NCORES):
                        nc.sync.dma_start(
                            xw_tbl2[_rr * SPLIT2:(_rr + 1) * SPLIT2, :],
                            xw_in2[:])
                else:
                    nc.gpsimd.collective_compute(
                        "AllGather", ALU.bypass,
                        replica_groups=[list(range(NCORES))],
                        ins=[xw_in2[:]], outs=[xw_tbl2[:]])

                # ---- gather machinery
                pair_tiles = {}

                def issue_pair_gathers(pi2):
                    pc2 = meta["pairs"][pi2]
                    pca2 = int(sum(capA[c] for c in pc2))
                    pcb2 = int(sum(capB[c] for c in pc2))
                    ioA2 = meta["pair_icolA"][pi2]
                    idxp2 = sp_.tile([128, (pca2 + pcb2) // 16], I16,
                                     tag="idxp", bufs=3,
                                     name=f"idxp_{li}_{pi2}")
                    nc.sync.dma_start(
                        idxp2[:],
                        T["idx_all"][:, ioA2:ioA2 + (pca2 + pcb2) // 16])
                    GA2 = gp.tile([128, pca2 // 128, HC], BF16, tag="GA",
                                  name=f"GA_{li}_{pi2}")
                    nc.gpsimd.dma_gather(
                        GA2[:], xw_tbl1[:], idxp2[:, 0:pca2 // 16],
                        pca2, pca2, HC, single_packet=False,
                        queue_num=(2 * pi2) % 4)
                    GB2 = gp.tile([128, pcb2 // 128, HC], BF16, tag="GB",
                                  name=f"GB_{li}_{pi2}")
                    nc.gpsimd.dma_gather(
                        GB2[:], xw_tbl2[:],
                        idxp2[:, pca2 // 16:(pca2 + pcb2) // 16],
                        pcb2, pcb2, HC, single_packet=False,
                        queue_num=(2 * pi2 + 1) % 4)
                    pair_tiles[pi2] = (GA2, GB2)

                issue_pair_gathers(0)
                issue_pair_gathers(1)

                # ---- pass B: k,v projections; kvs/ks accumulation
                kvsP = [ps.tile([128, 512], F32, space="PSUM",
                                tag=f"psb{h}", name=f"kvs{li}_{h}")
                        for h in range(H)]
                ksP = ps.tile([128, 8], F32, space="PSUM", tag="psb4",
                              name=f"ks{li}")
                _projtags[0] = ["psb5", "psb6", "psb7"]

                prev_acc = [None]

                def _emit_acc(kh_, vb_, t_):
                    for hh in range(H):
                        for half in range(2):
                            lhs = kh_[:, hh * HC + half * 128:
                                      hh * HC + half * 128 + 128]
                            nc.tensor.matmul(
                                kvsP[hh][:, half * HC:(half + 1) * HC],
                                lhsT=lhs, rhs=vb_[:, hh * HC:(hh + 1) * HC],
                                start=(t_ == 0), stop=(t_ == NT - 1))
                            nc.tensor.matmul(
                                ksP[:, 2 * hh + half:2 * hh + half + 1],
                                lhsT=lhs, rhs=maskbf[:, t_:t_ + 1],
                                start=(t_ == 0), stop=(t_ == NT - 1))

                for t in range(NT):
                    khat = stp.tile([128, 1024], BF16, tag="khat", bufs=3)
                    vsb = stp.tile([128, 1024], BF16, tag="vsb", bufs=3)
                    for nb in range(4):
                        kvp = proj_tile()
                        for k in range(4):
                            nc.tensor.matmul(
                                kvp[:], lhsT=Ht[t][:, k, :],
                                rhs=wkv_t[:, k, nb * 512:(nb + 1) * 512],
                                start=(k == 0), stop=(k == 3) and bkv is None)
                        if bkv is not None:
                            nc.tensor.matmul(
                                kvp[:], lhsT=onesrow[:],
                                rhs=bkv[li][:, nb * 512:(nb + 1) * 512],
                                start=False, stop=True)
                        if nb < 2:
                            ssk = smp.tile([128, 2], F32, tag="ssk")
                            for h2 in range(2):
                                sq = scp.tile([128, HC], BF16, tag="sq")
                                nc.scalar.activation(
                                    sq[:], kvp[:, h2 * HC:(h2 + 1) * HC],
                                    AF.Square, accum_out=ssk[:, h2:h2 + 1])
                            nrm = smp.tile([128, 2], F32, tag="nrmk")
                            nc.scalar.activation(nrm[:], ssk[:], AF.Sqrt,
                                                 bias=eps[:, :1])
                            rskm = smp.tile([128, 2], F32, tag="rskm")
                            nc.vector.reciprocal(rskm[:], nrm[:])
                            nc.vector.tensor_scalar(rskm[:], rskm[:],
                                                    maskc[:, t:t + 1], None,
                                                    op0=ALU.mult)
                            for h2 in range(2):
                                hh = nb * 2 + h2
                                if h2 == 0:
                                    nc.vector.tensor_scalar(
                                        khat[:, hh * HC:(hh + 1) * HC],
                                        kvp[:, h2 * HC:(h2 + 1) * HC],
                                        rskm[:, h2:h2 + 1], None,
                                        op0=ALU.mult)
                                else:
                                    nc.scalar.activation(
                                        khat[:, hh * HC:(hh + 1) * HC],
                                        kvp[:, h2 * HC:(h2 + 1) * HC],
                                        AF.Copy, scale=rskm[:, h2:h2 + 1])
                        else:
                            sl = slice((nb - 2) * 512, (nb - 1) * 512)
                            if nb == 2:
                                nc.vector.tensor_copy(vsb[:, sl], kvp[:])
                            else:
                                nc.scalar.activation(vsb[:, sl], kvp[:],
                                                     AF.Copy)
                    if prev_acc[0] is not None:
                        _emit_acc(*prev_acc[0])
                    prev_acc[0] = (khat, vsb, t)
                _emit_acc(*prev_acc[0])

                # ---- vs = hsum @ Wv  (+ nvalid * bv)
                hsbf = smp.tile([128, 4], BF16, tag="hsbf")
                nc.vector.tensor_copy(hsbf[:], hsumT[li][:])
                vsP = [ps.tile([1, 512], F32, space="PSUM", tag=f"psb{5 + i}",
                               name=f"vs{li}_{i}") for i in range(2)]
                for i in range(2):
                    for k in range(4):
                        nc.tensor.matmul(
                            vsP[i][:], lhsT=hsbf[:, k:k + 1],
                            rhs=wkv_t[:, k, 1024 + i * 512:1024 + (i + 1) * 512],
                            start=(k == 0), stop=(k == 3))

                # ---- flush kvs/ks/vs to cc_in, AllReduce
                cc_kvs = cc_in[:CCK].rearrange("(h p c) -> h p c", h=4, p=128)
                cc_vs = cc_in[CCK:].rearrange("(o c) -> o c", o=1)
                cco_kvs = cc_out[:CCK].rearrange("(h p c) -> h p c", h=4, p=128)
                cco_vs = cc_out[CCK:].rearrange("(o c) -> o c", o=1)
                for hh in range(H):
                    stg = stp.tile([128, 514], F32, tag="ccstage", bufs=2)
                    for half in range(2):
                        nc.vector.tensor_copy(
                            stg[:, half * 257:half * 257 + 256],
                            kvsP[hh][:, half * HC:(half + 1) * HC])
                        nc.vector.tensor_copy(
                            stg[:, half * 257 + 256:half * 257 + 257],
                            ksP[:, 2 * hh + half:2 * hh + half + 1])
                    nc.sync.dma_start(cc_kvs[hh], stg[:])
                vstg = cp.tile([1, 1024], F32, tag="vstage",
                               name=f"vstage{li}")
                for i in range(2):
                    nc.vector.tensor_copy(vstg[:, i * 512:(i + 1) * 512],
                                          vsP[i][:])
                if vsbias is not None:
                    nc.vector.tensor_tensor(vstg[:], vstg[:],
                                            vsbias[li:li + 1, :], op=ALU.add)
                nc.sync.dma_start(cc_vs, vstg[:])
                if single:
                    nc.sync.dma_start(cc_out[:], cc_in[:])
                else:
                    nc.gpsimd.collective_compute(
                        "AllReduce", ALU.add,
                        replica_groups=[list(range(NCORES))],
                        ins=[cc_in[:]], outs=[cc_out[:]])
                if debug:
                    nc.sync.dma_start(dbg["cc_dbg"][:], cc_out[:])

                # ---- load reduced stats: rhs tiles (0.25-scaled kvs, ks col)
                kvs_rhs = []
                for hh in range(H):
                    row = []
                    for half in range(2):
                        f32t = stp.tile([128, 257], F32, tag="ccload",
                                        bufs=2)
                        nc.sync.dma_start(
                            f32t[:], cco_kvs[hh][:, half * 257:(half + 1) * 257])
                        bft = cp.tile([128, 257], BF16, tag=f"kvsr{hh}_{half}",
                                      name=f"kvsr{li}_{hh}_{half}")
                        nc.vector.tensor_scalar(bft[:, 0:256], f32t[:, 0:256],
                                                0.25, None, op0=ALU.mult)
                        nc.vector.tensor_copy(bft[:, 256:257], f32t[:, 256:257])
                        row.append(bft)
                    kvs_rhs.append(row)
                vs_rhs = cp.tile([1, H, 257], BF16, tag="vsr", name=f"vsr{li}")
                nc.vector.memset(vs_rhs[:], 0)
                vrow = cp.tile([1, 1024], F32, tag="vsload",
                               name=f"vsload{li}")
                nc.sync.dma_start(vrow[:], cco_vs[:])
                for hh in range(H):
                    nc.vector.tensor_scalar(
                        vs_rhs[:1, hh, 0:256],
                        vrow[:, hh * 256:(hh + 1) * 256], 0.25, None,
                        op0=ALU.mult)

                # ---- pass 2: q, attention, GCN, epilogue
                _projtags[0] = ["psb4", "psb5", "psb6", "psb7"]
                pair_of = {}
                for pi, pc in enumerate(meta["pairs"]):
                    for j2, c in enumerate(pc):
                        pair_of[c] = (pi, j2)
                for t in range(NT):
                    ssq = smp.tile([128, H], F32, tag="ssq")
                    qhat = stp.tile([128, 1024], BF16, tag="qhat", bufs=3)
                    qchunks = []
                    for nb in range(2):
                        qp = proj_tile()
                        qchunks.append(qp)
                        for k in range(4):
                            nc.tensor.matmul(
                                qp[:], lhsT=Ht[t][:, k, :],
                                rhs=wq_t[:, k * 1024 + nb * 512:
                                            k * 1024 + (nb + 1) * 512],
                                start=(k == 0), stop=(k == 3) and bq_ is None)
                        if bq_ is not None:
                            nc.tensor.matmul(
                                qp[:], lhsT=onesrow[:],
                                rhs=bq_[li][:, nb * 512:(nb + 1) * 512],
                                start=False, stop=True)
                        for h2 in range(2):
                            hh = nb * 2 + h2
                            sq = scp.tile([128, HC], BF16, tag="sq")
                            nc.scalar.activation(
                                sq[:], qp[:, h2 * HC:(h2 + 1) * HC],
                                AF.Square, accum_out=ssq[:, hh:hh + 1])
                    nrmq = smp.tile([128, H], F32, tag="nrmq")
                    nc.scalar.activation(nrmq[:], ssq[:], AF.Sqrt,
                                         scale=16.0, bias=eps16[:, :1])
                    rsq = smp.tile([128, H], F32, tag="rsq")
                    nc.vector.reciprocal(rsq[:], nrmq[:])
                    for hh in range(H):
                        if hh % 2 == 0:
                            nc.vector.tensor_scalar(
                                qhat[:, hh * HC:(hh + 1) * HC],
                                qchunks[hh // 2][:, (hh % 2) * HC:(hh % 2 + 1) * HC],
                                rsq[:, hh:hh + 1], None, op0=ALU.mult)
                        else:
                            nc.scalar.activation(
                                qhat[:, hh * HC:(hh + 1) * HC],
                                qchunks[hh // 2][:, (hh % 2) * HC:(hh % 2 + 1) * HC],
                                AF.Copy, scale=rsq[:, hh:hh + 1])
                    qT = tp.tile([128, 8, 128], BF16, tag="qT", bufs=3)
                    nc.sync.dma_start_transpose(qT[:], qhat[:])

                    u = stp.tile([128, D2], BF16, tag="ucat", bufs=3)

                    # ---- GCN for chunk t
                    ca, cb = int(capA[t]), int(capB[t])
                    nk = int(nkt[t])
                    ko0 = meta["chunk_kcol"][t]
                    pi, j2 = pair_of[t]
                    if j2 == 0 and pi + 2 < npairs:
                        issue_pair_gathers(pi + 2)
                    pc = meta["pairs"][pi]
                    aoff = sum(int(capA[c]) // 128 for c in pc[:j2])
                    boff = sum(int(capB[c]) // 128 for c in pc[:j2])
                    GA, GB = pair_tiles[pi]
                    Sc = sp_.tile([128, nk * 128], mybir.dt.float8e4, tag="St",
                                  name=f"S_{li}_{t}")
                    nc.sync.dma_start(
                        Sc[:], T["s_all"][:, ko0 * 128:(ko0 + nk) * 128])
                    gcnP = gcn_tile()
                    for j in range(nk):
                        Gj = (GA[:, aoff + j, :] if j < ca // 128
                              else GB[:, boff + j - ca // 128, :])
                        nc.tensor.matmul(gcnP[:],
                                         lhsT=Sc[:, j * 128:(j + 1) * 128],
                                         rhs=Gj,
                                         start=(j == 0), stop=(j == nk - 1))
                    nc.scalar.activation(u[:, HC:], gcnP[:], AF.Copy,
                                         scale=dinvc[:, t:t + 1])
                    if debug:
                        nc.sync.dma_start(
                            dbg["x2_dbg"][:, t * HC:(t + 1) * HC], u[:, HC:])

                    # ---- attention numerator/denominator
                    for hh in range(H):
                        ndh = nd_tile(f"nd{li}_{t}_{hh}")
                        nc.tensor.matmul(ndh[:], lhsT=onesrow[:],
                                         rhs=vs_rhs[:1, hh, :],
                                         start=True, stop=False)
                        for kk in range(2):
                            nc.tensor.matmul(
                                ndh[:], lhsT=qT[:, 2 * hh + kk, :],
                                rhs=kvs_rhs[hh][kk][:],
                                start=False, stop=(kk == 1))
                        dtmp = smp.tile([128, 1], F32, tag="dtmp")
                        nc.vector.tensor_scalar(dtmp[:], ndh[:, 256:257],
                                                50000.0, None, op0=ALU.add)
                        rden = smp.tile([128, 1], F32, tag="rden")
                        nc.vector.reciprocal(rden[:], dtmp[:])
                        if hh == 0:
                            nc.scalar.activation(u[:, 0:HC], ndh[:, 0:256],
                                                 AF.Copy,
                                                 scale=rden[:, 0:1])
                        else:
                            nc.vector.scalar_tensor_tensor(
                                u[:, 0:HC], ndh[:, 0:256], rden[:, 0:1],
                                u[:, 0:HC], op0=ALU.mult, op1=ALU.add)
                    if debug:
                        nc.sync.dma_start(
                            dbg["x1_dbg"][:, t * HC:(t + 1) * HC], u[:, 0:HC])
                    # ---- epilogue (transposed)
                    epilogue(t, li + 1, u=u,
                             hacc=hsumT[li + 1] if li + 1 < L else None)
                if debug and li == 0:
                    for t in range(NT):
                        nc.sync.dma_start(
                            dbg["h1_dbg"][:, t * D2:(t + 1) * D2],
                            Ht[t][:].rearrange("p k j -> p (k j)"))

            # ---------------- final: out = h @ fc_out_w (+ bias)
            _projtags[0] = ["psb6", "psb7"]
            if meta["out_bias"]:
                fob = cp.tile([128, OUT], F32, tag="fcoutb")
                nc.sync.dma_start(fob[:], T["fcoutb"][:])
            for t in range(NT):
                op_ = proj_tile([128, OUT])
                for k in range(4):
                    nc.tensor.matmul(op_[:], lhsT=Ht[t][:, k, :],
                                     rhs=fcoutw[:, k * OUT:(k + 1) * OUT],
                                     start=(k == 0), stop=(k == 3))
                of = stp.tile([128, OUT], F32, tag="of")
                if meta["out_bias"]:
                    nc.vector.tensor_tensor(of[:], op_[:], fob[:],
                                            op=ALU.add)
                else:
                    nc.vector.tensor_copy(of[:], op_[:])
                nc.sync.dma_start(out_d[t * 128:(t + 1) * 128, :], of[:])

    nc.compile()
    return nc


# ------------------------------------------------------------------- runner
class _SpmdRunner:
    def __init__(self, nc, n_cores):
        install_neuronx_cc_hook()
        self.nc = nc
        self.n_cores = n_cores
        partition_name = (nc.partition_id_tensor.name
                          if nc.partition_id_tensor else None)
        in_names, out_names, out_avals = [], [], []
        for alloc in nc.m.functions[0].allocations:
            if not isinstance(alloc, mybir.MemoryLocationSet):
                continue
            name = alloc.memorylocations[0].name
            if alloc.kind == "ExternalInput":
                if name != partition_name:
                    in_names.append(name)
            elif alloc.kind == "ExternalOutput":
                out_names.append(name)
                out_avals.append(jax.core.ShapedArray(
                    tuple(alloc.tensor_shape), mybir.dt.np(alloc.dtype)))
        self.in_names, self.out_names, self.out_avals = \
            in_names, out_names, out_avals
        n_params = len(in_names)
        all_in = list(in_names) + list(out_names)
        if partition_name is not None:
            all_in.append(partition_name)

        def _body(*args):
            operands = list(args)
            if partition_name is not None:
                operands.append(bass2jax.partition_id_tensor())
            outs = _bass_exec_p.bind(
                *operands, out_avals=tuple(out_avals),
                in_names=tuple(all_in), out_names=tuple(out_names),
                lowering_input_output_aliases=(),
                sim_require_finite=True, sim_require_nnan=True, nc=nc)
            return tuple(outs)

        devices = jax.devices()[:n_cores]
        self.mesh = Mesh(np.asarray(devices), ("core",))
        in_specs = (PartitionSpec("core"),) * (n_params + len(out_names))
        out_specs = (PartitionSpec("core"),) * len(out_names)
        self.fn = jax.jit(
            shard_map(_body, mesh=self.mesh, in_specs=in_specs,
                      out_specs=out_specs, check_rep=False),
            keep_unused=True)
        self._dev_in = None

    def set_inputs(self, in_maps):
        n = self.n_cores
        concat = [np.concatenate([np.asarray(in_maps[c][nm]) for c in range(n)],
                                 axis=0) for nm in self.in_names]
        for av in self.out_avals:
            concat.append(np.zeros((n * av.shape[0], *av.shape[1:]), av.dtype))
        sh = jax.sharding.NamedSharding(self.mesh, PartitionSpec("core"))
        self._dev_in = [jax.device_put(a, sh) for a in concat]

    def run(self):
        outs = self.fn(*self._dev_in)
        jax.block_until_ready(outs)
        return [{nm: np.asarray(outs[i]).reshape(
                    self.n_cores, *self.out_avals[i].shape)[c]
                 for i, nm in enumerate(self.out_names)}
                for c in range(self.n_cores)]

    def time_ns(self, iters=10, warmup=2):
        for _ in range(warmup):
            jax.block_until_ready(self.fn(*self._dev_in))
        ts = []
        for _ in range(iters):
            t0 = time.perf_counter_ns()
            jax.block_until_ready(self.fn(*self._dev_in))
            ts.append(time.perf_counter_ns() - t0)
        return min(ts), sorted(ts)[len(ts) // 2]


_CACHE = {}


def _get_runner(meta, debug=False):
    key = (tuple(meta["capA"]), tuple(meta["capB"]), meta["qkv_bias"],
           meta["out_bias"], debug)
    if key not in _CACHE:
        nc = _build_nc(meta, debug=debug)
        _CACHE[key] = _SpmdRunner(nc, NCORES)
    return _CACHE[key]


def kernel(x, edge_index, batch, fc0_w, fc0_b, wq, bq, wk, bk, wv, bv,
           gcn_w, gcn_b, bn_gamma, bn_beta, bn_mean, bn_var,
           fc_out_w, fc_out_b, _debug=False, _return_runner=False):
    in_maps, meta = _host_prep(
        x, edge_index, fc0_w, fc0_b, wq, bq, wk, bk, wv, bv,
        gcn_w, gcn_b, bn_gamma, bn_beta, bn_mean, bn_var,
        fc_out_w, fc_out_b)
    runner = _get_runner(meta, debug=_debug)
    runner.set_inputs(in_maps)
    results = runner.run()
    out = np.concatenate([results[c]["out"] for c in range(NCORES)], axis=0)
    out = out[:N].astype(np.float32)
    if _return_runner:
        return out, runner, results
    return out
